# revision 28
# baseline (speedup 1.0000x reference)
"""ChaRNN LSTM (teacher forcing) Trainium2 Bass kernel.

Structure (data-parallel over batch, 64 rows/core on 8 cores):
  - Teacher forcing means the LSTM input at step t is [f_pool[:,t,:], gt[:,t-1,:]]
    which is fully known ahead of time, so the input projection X @ W_x for all
    32 steps is a big batched matmul (phase A).  Only h @ W_h is sequential
    (phase B).  The argmax/one-hot head is deferred and batched (phase C).
  - Matmuls use fp16 hi/lo 3-pass splits (3 cyc/row total, fp32-grade\n    accuracy, validated 0 argmax flips) - single-pass bf16/fp16 flips
    argmaxes (one-hot output is graded, top-2 logit gaps go down to 6.6e-6).
  - Gate activations use tanh only (4 ULP) via sigmoid(x) = 0.5*tanh(x/2)+0.5;
    the 0.5 pre-scale of the i/f/o gate columns is folded into the weights on
    the host, so one ACT pass computes tanh over the whole 2048-wide gate row.
  - Phase A is emitted interleaved with the recurrence so the tensor engine
    fills the pointwise-tail gaps of each step with input-projection matmuls.
  - Recurrence matmuls pack pairs of K-chunks into PE column halves
    (tile_position (0,0)/(0,64)) since batch=64 only fills half the array;
    measured 1.84x over the unpacked form.
"""

import os
import numpy as np

B, T, DEPTH = 512, 32, 512
RNN, NCC = 512, 128
DIN = DEPTH + NCC            # 640
G4 = 4 * RNN                 # 2048
NCORES = 8
BS = B // NCORES             # 64 batch rows per core
ROWS = T * BS                # 2048 (t-major: r = t*BS + b)

_PROGRAM = None


def _build_program():
    import concourse.bass as bass
    import concourse.tile as tile
    from concourse import bacc, mybir
    from concourse.masks import make_identity

    f32 = mybir.dt.float32
    nc = bacc.Bacc(None)

    f16 = mybir.dt.float16

    xth_d = nc.dram_tensor("xth", [DIN, ROWS], f16, kind="ExternalInput")
    xtl_d = nc.dram_tensor("xtl", [DIN, ROWS], f16, kind="ExternalInput")
    wxh_d = nc.dram_tensor("wxh", [DIN, G4], f16, kind="ExternalInput")
    wxl_d = nc.dram_tensor("wxl", [DIN, G4], f16, kind="ExternalInput")
    whh_d = nc.dram_tensor("whh", [RNN, G4], f16, kind="ExternalInput")
    whl_d = nc.dram_tensor("whl", [RNN, G4], f16, kind="ExternalInput")
    biasw_d = nc.dram_tensor("biasw", [2, G4], f16, kind="ExternalInput")
    smwh_d = nc.dram_tensor("smwh", [RNN, NCC], f16, kind="ExternalInput")
    smwl_d = nc.dram_tensor("smwl", [RNN, NCC], f16, kind="ExternalInput")
    smbb_d = nc.dram_tensor("smbb", [128, NCC], f32, kind="ExternalInput")
    revidx_d = nc.dram_tensor("revidx", [128, NCC], f32, kind="ExternalInput")

    onehot_d = nc.dram_tensor("onehot", [ROWS, NCC], f32, kind="ExternalOutput")
    h_d = nc.dram_tensor("h_out", [BS, RNN], f32, kind="ExternalOutput")
    c_d = nc.dram_tensor("c_out", [BS, RNN], f32, kind="ExternalOutput")

    xth_r = xth_d.rearrange("(kc p) r -> p kc r", p=128)
    xtl_r = xtl_d.rearrange("(kc p) r -> p kc r", p=128)
    wxh_r = wxh_d.rearrange("(kc p) g -> p kc g", p=128)
    wxl_r = wxl_d.rearrange("(kc p) g -> p kc g", p=128)
    whh_r = whh_d.rearrange("(kc p) g -> p kc g", p=128)
    whl_r = whl_d.rearrange("(kc p) g -> p kc g", p=128)
    smwh_r = smwh_d.rearrange("(kc p) n -> p kc n", p=128)
    smwl_r = smwl_d.rearrange("(kc p) n -> p kc n", p=128)

    AluOp = mybir.AluOpType
    Act = mybir.ActivationFunctionType

    with tile.TileContext(nc) as tc:
        with (
            tc.tile_pool(name="persist", bufs=1) as pp,
            tc.tile_pool(name="weights", bufs=1) as wp,
            tc.tile_pool(name="astage", bufs=2) as ap_,
            tc.tile_pool(name="axt", bufs=2) as axt,
            tc.tile_pool(name="bwork", bufs=2) as bp,
            tc.tile_pool(name="bwork1", bufs=1) as bp1,
            tc.tile_pool(name="aps", bufs=4, space="PSUM") as aps,
            tc.tile_pool(name="bps", bufs=1, space="PSUM") as bps,
        ):
            ident = pp.tile([128, 128], f16)
            make_identity(nc, ident[:])
            # h state, fp16 hi/lo pair, transposed; split by rnn half so the
            # next step's first matmul pair only waits on half 0.
            # HTxx[:, kc, s*64+b] = h_(s-1)[b, ...]; block s=0 is h0 = 0.
            HTH = (pp.tile([128, 2, (T + 1) * BS], f16, name="hth0"),
                   pp.tile([128, 2, (T + 1) * BS], f16, name="hth1"))
            HTL = (pp.tile([128, 2, (T + 1) * BS], f16, name="htl0"),
                   pp.tile([128, 2, (T + 1) * BS], f16, name="htl1"))
            for tl in (*HTH, *HTL):
                nc.gpsimd.memset(tl[:, :, 0:BS], 0.0)

            wxh_t = wp.tile([128, 5, G4], f16)
            wxl_t = wp.tile([128, 5, G4], f16)
            whh_t = wp.tile([128, 4, G4], f16)
            whl_t = wp.tile([128, 4, G4], f16)
            for kc in range(5):
                nc.sync.dma_start(wxh_t[:, kc, :], wxh_r[:, kc, :])
                nc.sync.dma_start(wxl_t[:, kc, :], wxl_r[:, kc, :])
            for kc in range(4):
                nc.sync.dma_start(whh_t[:, kc, :], whh_r[:, kc, :])
                nc.sync.dma_start(whl_t[:, kc, :], whl_r[:, kc, :])
            biasw_t = wp.tile([2, 4, 512], f16)
            nc.sync.dma_start(biasw_t[:], biasw_d.rearrange("k (n x) -> k n x", n=4))
            ones_t = wp.tile([2, 128], f16)
            nc.gpsimd.memset(ones_t[:], 1.0)
            smwh_t = wp.tile([128, 4, NCC], f16)
            nc.sync.dma_start(smwh_t[:], smwh_r[:])
            smwl_t = wp.tile([128, 4, NCC], f16)
            nc.sync.dma_start(smwl_t[:], smwl_r[:])
            smb_t = wp.tile([128, NCC], f32)
            nc.sync.dma_start(smb_t[:], smbb_d[:])
            revidx_t = wp.tile([128, NCC], f32)
            nc.sync.dma_start(revidx_t[:], revidx_d[:])
            logits = pp.tile([128, 16, NCC], f32)

            c_cur = bp.tile([BS, RNN], f32, tag="c")
            nc.gpsimd.memset(c_cur[:], 0.0)

            # Z stage ring: one tile holds one rc-block (128 rows x 4 gates
            # x 512) of the input projection = 2 timesteps worth.  Split into
            # half-blocks (2 gate-chunks) so the PE filler between a step's
            # recurrence matmuls and its transposes stays within PSUM budget;
            # psum->stage copies go on the idle Scalar engine afterwards.
            def emit_phaseA_half_mms(rc, hb, xth_rc, xtl_rc):
                pss = []
                for n in (2 * hb, 2 * hb + 1):
                    ps = aps.tile([128, 512], f32, tag="aps")
                    for kc in range(5):
                        for pi, (lh, rh) in enumerate((
                                (xth_rc, wxh_t), (xth_rc, wxl_t),
                                (xtl_rc, wxh_t))):
                            nc.tensor.matmul(
                                ps[:], lh[:, kc, :], rh[:, kc, bass.ts(n, 512)],
                                start=(kc == 0 and pi == 0), stop=False,
                            )
                    # += bias (hi+lo fp16 rows against a ones stationary)
                    nc.tensor.matmul(ps[:], ones_t[:], biasw_t[:, n, :],
                                     start=False, stop=True)
                    pss.append((n, ps))
                return pss

            def emit_phaseA_half_copies(stage, pss):
                for n, ps in pss:
                    nc.scalar.copy(stage[:, n, :], ps[:])

            def emit_phaseC_block(rc):
                pl = bps.tile([128, NCC], f32, tag="zpa")
                first, last = (0, 0), (3, 2)
                for kc in range(4):
                    hh = HTH[kc // 2][:, kc % 2, bass.ds(BS + rc * 128, 128)]
                    hl = HTL[kc // 2][:, kc % 2, bass.ds(BS + rc * 128, 128)]
                    for pi, (lh, rh) in enumerate((
                            (hh, smwh_t), (hh, smwl_t), (hl, smwh_t))):
                        nc.tensor.matmul(
                            pl[:], lh, rh[:, kc, :],
                            start=((kc, pi) == first), stop=((kc, pi) == last),
                        )
                nc.vector.tensor_tensor(out=logits[:, rc, :], in0=pl[:],
                                        in1=smb_t[:], op=AluOp.add)
                rm1 = pp.tile([128, 16], f32, name=f"rm1_{rc}", tag="rm1")
                nc.vector.tensor_reduce(rm1[:, 0:1], logits[:, rc, :],
                                        axis=mybir.AxisListType.X, op=AluOp.max)
                nc.vector.tensor_scalar(out=logits[:, rc, :], in0=logits[:, rc, :],
                                        scalar1=rm1[:, 0:1], scalar2=None,
                                        op0=AluOp.is_equal)
                nc.vector.tensor_tensor(out=logits[:, rc, :], in0=logits[:, rc, :],
                                        in1=revidx_t[:], op=AluOp.mult)
                nc.vector.tensor_reduce(rm1[:, 1:2], logits[:, rc, :],
                                        axis=mybir.AxisListType.X, op=AluOp.max)
                nc.vector.tensor_scalar(out=logits[:, rc, :], in0=revidx_t[:],
                                        scalar1=rm1[:, 1:2], scalar2=None,
                                        op0=AluOp.is_equal)
                nc.sync.dma_start(onehot_d[bass.ts(rc, 128), :], logits[:, rc, :])

            stages = {}
            h2 = None

            def new_ablock(rc):
                xth_rc = axt.tile([128, 5, 128], f16, tag="xth")
                nc.sync.dma_start(xth_rc[:], xth_r[:, :, rc * 128:(rc + 1) * 128])
                xtl_rc = axt.tile([128, 5, 128], f16, tag="xtl")
                nc.sync.dma_start(xtl_rc[:], xtl_r[:, :, rc * 128:(rc + 1) * 128])
                stage = ap_.tile([128, 4, 512], f32, tag="stage")
                return [rc, stage, xth_rc, xtl_rc]

            # prelude: first two rc blocks fully materialized
            for rc in (0, 1):
                blk = new_ablock(rc)
                for hb in range(2):
                    pss = emit_phaseA_half_mms(rc, hb, blk[2], blk[3])
                    emit_phaseA_half_copies(blk[1], pss)
                stages[rc] = blk[1]
            cur_blk = new_ablock(2)
            nxt_blk = new_ablock(3)
            next_half = 0
            pending = None  # (blk, next_half_flag, pss) copies deferred to next step

            C_SCHED = {28: [0, 1, 2, 3], 29: [4, 5, 6, 7],
                       30: [8, 9, 10, 14], 31: [11, 12, 13]}

            for t in range(T):
                # deferred phase-A psum->stage copies from the previous step's
                # filler run early in the Scalar stream, freeing psum before
                # this step's filler matmuls need it
                if pending is not None:
                    pblk, phalf, ppss = pending
                    emit_phaseA_half_copies(pblk[1], ppss)
                    if phalf == 1:
                        stages[pblk[0]] = pblk[1]
                        cur_blk = nxt_blk
                        nxt_blk = (new_ablock(pblk[0] + 2)
                                   if pblk[0] + 2 <= 15 else None)
                        next_half = 0
                    else:
                        next_half = 1
                    pending = None

                rc, half = divmod(t, 2)
                zin = stages[rc][bass.ds(half * BS, BS), :, :]  # [64, 4, 512]

                # ---- recurrence matmuls for step t ----
                # gates: 0=i, 1=f (zpa), 2=g, 3=o (zpb)
                zpa = bps.tile([128, 2, 512], f32, tag="zpa")
                zpb = bps.tile([128, 2, 512], f32, tag="zpb")
                for pi, (ka, kb) in enumerate(((0, 1), (2, 3))):
                    hha = HTH[ka // 2][:, ka % 2, bass.ds(t * BS, BS)]
                    hla = HTL[ka // 2][:, ka % 2, bass.ds(t * BS, BS)]
                    hhb = HTH[kb // 2][:, kb % 2, bass.ds(t * BS, BS)]
                    hlb = HTL[kb // 2][:, kb % 2, bass.ds(t * BS, BS)]
                    for n in range(4):
                        ps = zpa if n < 2 else zpb
                        g = n % 2
                        wsl = bass.ts(n, 512)
                        for qi, (la, lb, rh) in enumerate((
                                (hha, hhb, whh_t), (hha, hhb, whl_t),
                                (hla, hlb, whh_t))):
                            st = (pi == 0 and qi == 0)
                            sp = (pi == 1 and qi == 2)
                            nc.tensor.matmul(
                                ps[0:64, g, :], la, rh[:, ka, wsl],
                                start=st, stop=sp, tile_position=(0, 0),
                            )
                            nc.tensor.matmul(
                                ps[64:128, g, :], lb, rh[:, kb, wsl],
                                start=st, stop=sp, tile_position=(0, 64),
                            )

                # ---- PE filler emitted before the pointwise/transposes so the
                # in-order tensor stream works on it while the gate chain runs
                if cur_blk is not None:
                    pending = (cur_blk, next_half,
                               emit_phaseA_half_mms(cur_blk[0], next_half,
                                                    cur_blk[2], cur_blk[3]))
                for crc in C_SCHED.get(t, []):
                    emit_phaseC_block(crc)

                # ---- gate pointwise, h update, transposed fp16 hi/lo state ----
                za = bp.tile([BS, 4, 512], f32, tag="za")
                sif = bp1.tile([BS, 2, 512], f32, tag="sif")
                so = bp1.tile([BS, 512], f32, tag="so")
                m1 = bp1.tile([BS, 512], f32, tag="m1")
                m2 = bp1.tile([BS, 512], f32, tag="m2")
                c_new = bp.tile([BS, RNN], f32, tag="c")
                tc2 = bp1.tile([BS, 512], f32, tag="tc2")
                h2 = bp1.tile([BS, RNN], f32, tag="h2")
                h2h = bp1.tile([BS, RNN], f16, tag="h2h")
                h2l = bp1.tile([BS, RNN], f16, tag="h2l")
                trp = bps.tile([128, 8, 64], f16, tag="zpb")

                for hf in range(2):  # rnn halves: cols hf*256:(hf+1)*256
                    hs = bass.ds(hf * 256, 256)
                    # z' = zp_lo + zin, then += zp_hi  (one PSUM input per op)
                    nc.vector.tensor_tensor(out=za[:, 0:2, hs], in0=zpa[0:64, :, hs],
                                            in1=zin[:, 0:2, hs], op=AluOp.add)
                    nc.vector.tensor_tensor(out=za[:, 0:2, hs], in0=za[:, 0:2, hs],
                                            in1=zpa[64:128, :, hs], op=AluOp.add)
                    nc.vector.tensor_tensor(out=za[:, 2:4, hs], in0=zpb[0:64, :, hs],
                                            in1=zin[:, 2:4, hs], op=AluOp.add)
                    nc.vector.tensor_tensor(out=za[:, 2:4, hs], in0=za[:, 2:4, hs],
                                            in1=zpb[64:128, :, hs], op=AluOp.add)
                    # za = tanh(z'); i/f/o columns pre-scaled by 0.5 on host
                    nc.scalar.activation(za[:, :, hs], za[:, :, hs], Act.Tanh)
                    nc.vector.tensor_scalar(out=sif[:, :, hs], in0=za[:, 0:2, hs],
                                            scalar1=0.5, scalar2=0.5,
                                            op0=AluOp.mult, op1=AluOp.add)
                    nc.gpsimd.tensor_scalar(out=so[:, hs], in0=za[:, 3, hs],
                                            scalar1=0.5, scalar2=0.5,
                                            op0=AluOp.mult, op1=AluOp.add)
                    nc.gpsimd.tensor_tensor(out=m1[:, hs], in0=sif[:, 1, hs],
                                            in1=c_cur[:, hs], op=AluOp.mult)
                    nc.vector.tensor_tensor(out=m2[:, hs], in0=sif[:, 0, hs],
                                            in1=za[:, 2, hs], op=AluOp.mult)
                    nc.vector.tensor_tensor(out=c_new[:, hs], in0=m1[:, hs],
                                            in1=m2[:, hs], op=AluOp.add)
                    nc.scalar.activation(tc2[:, hs], c_new[:, hs], Act.Tanh)
                    nc.vector.tensor_tensor(out=h2[:, hs], in0=so[:, hs],
                                            in1=tc2[:, hs], op=AluOp.mult)
                    # fp16 hi/lo split of h2 for the next step's matmuls
                    nc.vector.tensor_copy(h2h[:, hs], h2[:, hs])
                    nc.vector.tensor_tensor(out=h2l[:, hs], in0=h2[:, hs],
                                            in1=h2h[:, hs], op=AluOp.subtract)
                    for kc in (2 * hf, 2 * hf + 1):
                        nc.tensor.transpose(trp[:, kc, :], h2h[:, bass.ts(kc, 128)],
                                            ident[0:64, 0:64])
                        nc.tensor.transpose(trp[:, 4 + kc, :],
                                            h2l[:, bass.ts(kc, 128)],
                                            ident[0:64, 0:64])
                    nc.vector.tensor_copy(
                        HTH[hf][:, :, bass.ds((t + 1) * BS, BS)],
                        trp[:, 2 * hf:2 * hf + 2, :])
                    nc.vector.tensor_copy(
                        HTL[hf][:, :, bass.ds((t + 1) * BS, BS)],
                        trp[:, 4 + 2 * hf:4 + 2 * hf + 2, :])
                c_cur = c_new

            if pending is not None:
                emit_phaseA_half_copies(pending[0][1], pending[2])
            emit_phaseC_block(15)
            nc.sync.dma_start(h_d[:], h2[:])
            nc.sync.dma_start(c_d[:], c_cur[:])

    nc.finalize()
    return nc


def _get_program():
    global _PROGRAM
    if _PROGRAM is None:
        _PROGRAM = _build_program()
    return _PROGRAM


def _prep_inputs(f_pool, ground_truth, kernel, rec_kernel, bias, softmax_w,
                 softmax_b):
    def split16(x):
        hi = x.astype(np.float16)
        lo = (x - hi.astype(np.float32)).astype(np.float16)
        return np.ascontiguousarray(hi), np.ascontiguousarray(lo)

    # fold the tanh half-angle pre-scale of gates i, f, o into the weights
    col_scale = np.ones((G4,), np.float32)
    col_scale[0 * RNN:2 * RNN] = 0.5     # i, f
    col_scale[3 * RNN:4 * RNN] = 0.5     # o
    wxh, wxl = split16(kernel * col_scale[None, :])
    whh, whl = split16(rec_kernel * col_scale[None, :])
    bias_s = (bias * col_scale).astype(np.float32)
    bh, bl = split16(bias_s[None, :])
    biasw = np.ascontiguousarray(np.concatenate([bh, bl], axis=0))
    smbb = np.ascontiguousarray(np.tile(softmax_b[None, :].astype(np.float32),
                                        (128, 1)))
    revidx = np.ascontiguousarray(
        np.tile((NCC - np.arange(NCC, dtype=np.float32))[None, :], (128, 1)))
    smwh, smwl = split16(softmax_w.astype(np.float32))

    in_maps = []
    for c in range(NCORES):
        fp = f_pool[c * BS:(c + 1) * BS]          # [64, 32, 512]
        gt = ground_truth[c * BS:(c + 1) * BS]    # [64, 32, 128]
        prev = np.zeros_like(gt)
        prev[:, 1:] = gt[:, :-1]
        fpT = np.ascontiguousarray(fp).transpose(2, 1, 0).reshape(DEPTH, ROWS)
        prT = np.ascontiguousarray(prev).transpose(2, 1, 0).reshape(NCC, ROWS)
        xt = np.concatenate([fpT, prT], axis=0)
        xth, xtl = split16(xt)
        in_maps.append({
            "xth": xth, "xtl": xtl, "wxh": wxh, "wxl": wxl,
            "whh": whh, "whl": whl, "biasw": biasw,
            "smwh": smwh, "smwl": smwl, "smbb": smbb, "revidx": revidx,
        })
    return in_maps


def _install_ntff_shim():
    """Register the axon NTFF profiling hook the image's antenv lacks."""
    import contextlib, ctypes, sys, types
    try:
        import antenv
    except ImportError:
        return
    if getattr(antenv, "axon_hooks", None) is not None:
        return
    state = {}
    mod = types.ModuleType("antenv.axon_hooks")
    mod.set_axon_ntff_profile_hook = lambda h: state.update(h=h)
    mod.get_axon_ntff_profile_hook = lambda: state.get("h")
    sys.modules["antenv.axon_hooks"] = mod
    antenv.axon_hooks = mod
    try:
        lib = ctypes.CDLL("/opt/axon/libaxon_pjrt.so")
    except OSError:
        return
    if not hasattr(lib, "axon_start_nrt_profile"):
        return
    lib.axon_start_nrt_profile.argtypes = [ctypes.POINTER(ctypes.c_int64),
                                           ctypes.c_size_t]
    lib.axon_start_nrt_profile.restype = ctypes.c_int64
    lib.axon_stop_nrt_profile.argtypes = [ctypes.c_char_p]
    lib.axon_stop_nrt_profile.restype = ctypes.c_int64

    @contextlib.contextmanager
    def _hook(output_dir, device_ids):
        import jax
        jax.devices()
        if device_ids:
            ids = (ctypes.c_int64 * len(device_ids))(*device_ids)
            rc = lib.axon_start_nrt_profile(ids, len(device_ids))
        else:
            rc = lib.axon_start_nrt_profile(None, 0)
        if rc != 0:
            raise RuntimeError(f"axon_start_nrt_profile rc={rc}")
        try:
            yield
        finally:
            n = lib.axon_stop_nrt_profile(str(output_dir).encode())
            if n < 0:
                raise RuntimeError(f"axon_stop_nrt_profile rc={n}")

    mod.set_axon_ntff_profile_hook(_hook)


def _run(in_maps, trace=False):
    from concourse.bass_utils import run_bass_kernel_spmd
    if trace:
        _install_ntff_shim()
    nc = _get_program()
    return run_bass_kernel_spmd(nc, in_maps, list(range(NCORES)), trace=trace)


def kernel(f_pool, ground_truth, kernel, rec_kernel, bias, softmax_w,
           softmax_b):
    f_pool = np.asarray(f_pool, np.float32)
    ground_truth = np.asarray(ground_truth, np.float32)
    in_maps = _prep_inputs(f_pool, ground_truth, np.asarray(kernel, np.float32),
                           np.asarray(rec_kernel, np.float32),
                           np.asarray(bias, np.float32),
                           np.asarray(softmax_w, np.float32),
                           np.asarray(softmax_b, np.float32))
    trace = bool(int(os.environ.get("KERNEL_TRACE", "0")))
    res = _run(in_maps, trace=trace)
    if trace and res.exec_time_ns is not None:
        print(f"HW exec time: {res.exec_time_ns} ns")

    seq = np.empty((B, T, NCC), np.float32)
    h = np.empty((B, RNN), np.float32)
    c = np.empty((B, RNN), np.float32)
    for ci in range(NCORES):
        r = res.results[ci]
        seq[ci * BS:(ci + 1) * BS] = (
            r["onehot"].reshape(T, BS, NCC).transpose(1, 0, 2))
        h[ci * BS:(ci + 1) * BS] = r["h_out"]
        c[ci * BS:(ci + 1) * BS] = r["c_out"]
    return (seq, h, c)


# revision 29
# speedup vs baseline: 1.0028x; 1.0028x over previous
"""ChaRNN LSTM (teacher forcing) Trainium2 Bass kernel.

Structure (data-parallel over batch, 64 rows/core on 8 cores):
  - Teacher forcing means the LSTM input at step t is [f_pool[:,t,:], gt[:,t-1,:]]
    which is fully known ahead of time, so the input projection X @ W_x for all
    32 steps is a big batched matmul (phase A).  Only h @ W_h is sequential
    (phase B).  The argmax/one-hot head is deferred and batched (phase C).
  - Matmuls use fp16 hi/lo 3-pass splits (3 cyc/row total, fp32-grade\n    accuracy, validated 0 argmax flips) - single-pass bf16/fp16 flips
    argmaxes (one-hot output is graded, top-2 logit gaps go down to 6.6e-6).
  - Gate activations use tanh only (4 ULP) via sigmoid(x) = 0.5*tanh(x/2)+0.5;
    the 0.5 pre-scale of the i/f/o gate columns is folded into the weights on
    the host, so one ACT pass computes tanh over the whole 2048-wide gate row.
  - Phase A is emitted interleaved with the recurrence so the tensor engine
    fills the pointwise-tail gaps of each step with input-projection matmuls.
  - Recurrence matmuls pack pairs of K-chunks into PE column halves
    (tile_position (0,0)/(0,64)) since batch=64 only fills half the array;
    measured 1.84x over the unpacked form.
"""

import os
import numpy as np

B, T, DEPTH = 512, 32, 512
RNN, NCC = 512, 128
DIN = DEPTH + NCC            # 640
G4 = 4 * RNN                 # 2048
NCORES = 8
BS = B // NCORES             # 64 batch rows per core
ROWS = T * BS                # 2048 (t-major: r = t*BS + b)

_PROGRAM = None


def _build_program():
    import concourse.bass as bass
    import concourse.tile as tile
    from concourse import bacc, mybir
    from concourse.masks import make_identity

    f32 = mybir.dt.float32
    nc = bacc.Bacc(None)

    f16 = mybir.dt.float16

    xth_d = nc.dram_tensor("xth", [DIN, ROWS], f16, kind="ExternalInput")
    xtl_d = nc.dram_tensor("xtl", [DIN, ROWS], f16, kind="ExternalInput")
    wxh_d = nc.dram_tensor("wxh", [DIN, G4], f16, kind="ExternalInput")
    wxl_d = nc.dram_tensor("wxl", [DIN, G4], f16, kind="ExternalInput")
    whh_d = nc.dram_tensor("whh", [RNN, G4], f16, kind="ExternalInput")
    whl_d = nc.dram_tensor("whl", [RNN, G4], f16, kind="ExternalInput")
    biasw_d = nc.dram_tensor("biasw", [2, G4], f16, kind="ExternalInput")
    smwh_d = nc.dram_tensor("smwh", [RNN, NCC], f16, kind="ExternalInput")
    smwl_d = nc.dram_tensor("smwl", [RNN, NCC], f16, kind="ExternalInput")
    smbb_d = nc.dram_tensor("smbb", [128, NCC], f32, kind="ExternalInput")
    revidx_d = nc.dram_tensor("revidx", [128, NCC], f32, kind="ExternalInput")

    onehot_d = nc.dram_tensor("onehot", [ROWS, NCC], f32, kind="ExternalOutput")
    h_d = nc.dram_tensor("h_out", [BS, RNN], f32, kind="ExternalOutput")
    c_d = nc.dram_tensor("c_out", [BS, RNN], f32, kind="ExternalOutput")

    xth_r = xth_d.rearrange("(kc p) r -> p kc r", p=128)
    xtl_r = xtl_d.rearrange("(kc p) r -> p kc r", p=128)
    wxh_r = wxh_d.rearrange("(kc p) g -> p kc g", p=128)
    wxl_r = wxl_d.rearrange("(kc p) g -> p kc g", p=128)
    whh_r = whh_d.rearrange("(kc p) g -> p kc g", p=128)
    whl_r = whl_d.rearrange("(kc p) g -> p kc g", p=128)
    smwh_r = smwh_d.rearrange("(kc p) n -> p kc n", p=128)
    smwl_r = smwl_d.rearrange("(kc p) n -> p kc n", p=128)

    AluOp = mybir.AluOpType
    Act = mybir.ActivationFunctionType

    with tile.TileContext(nc) as tc:
        with (
            tc.tile_pool(name="persist", bufs=1) as pp,
            tc.tile_pool(name="weights", bufs=1) as wp,
            tc.tile_pool(name="astage", bufs=3) as ap_,
            tc.tile_pool(name="axt", bufs=2) as axt,
            tc.tile_pool(name="bwork", bufs=2) as bp,
            tc.tile_pool(name="bwork1", bufs=1) as bp1,
            tc.tile_pool(name="aps", bufs=3, space="PSUM") as aps,
            tc.tile_pool(name="bps", bufs=1, space="PSUM") as bps,
            tc.tile_pool(name="clps", bufs=1, space="PSUM") as clps,
        ):
            ident = pp.tile([128, 128], f16)
            make_identity(nc, ident[:])
            # h state, fp16 hi/lo pair, transposed; split by rnn half so the
            # next step's first matmul pair only waits on half 0.
            # HTxx[:, kc, s*64+b] = h_(s-1)[b, ...]; block s=0 is h0 = 0.
            HTH = (pp.tile([128, 2, (T + 1) * BS], f16, name="hth0"),
                   pp.tile([128, 2, (T + 1) * BS], f16, name="hth1"))
            HTL = (pp.tile([128, 2, (T + 1) * BS], f16, name="htl0"),
                   pp.tile([128, 2, (T + 1) * BS], f16, name="htl1"))
            for tl in (*HTH, *HTL):
                nc.gpsimd.memset(tl[:, :, 0:BS], 0.0)

            wxh_t = wp.tile([128, 5, G4], f16)
            wxl_t = wp.tile([128, 5, G4], f16)
            whh_t = wp.tile([128, 4, G4], f16)
            whl_t = wp.tile([128, 4, G4], f16)
            for kc in range(5):
                nc.sync.dma_start(wxh_t[:, kc, :], wxh_r[:, kc, :])
                nc.sync.dma_start(wxl_t[:, kc, :], wxl_r[:, kc, :])
            for kc in range(4):
                nc.sync.dma_start(whh_t[:, kc, :], whh_r[:, kc, :])
                nc.sync.dma_start(whl_t[:, kc, :], whl_r[:, kc, :])
            biasw_t = wp.tile([2, 4, 512], f16)
            nc.sync.dma_start(biasw_t[:], biasw_d.rearrange("k (n x) -> k n x", n=4))
            ones_t = wp.tile([2, 128], f16)
            nc.gpsimd.memset(ones_t[:], 1.0)
            smwh_t = wp.tile([128, 4, NCC], f16)
            nc.sync.dma_start(smwh_t[:], smwh_r[:])
            smwl_t = wp.tile([128, 4, NCC], f16)
            nc.sync.dma_start(smwl_t[:], smwl_r[:])
            smb_t = wp.tile([128, NCC], f32)
            nc.sync.dma_start(smb_t[:], smbb_d[:])
            revidx_t = wp.tile([128, NCC], f32)
            nc.sync.dma_start(revidx_t[:], revidx_d[:])
            logits = pp.tile([128, 16, NCC], f32)

            c_cur = bp.tile([BS, RNN], f32, tag="c")
            nc.gpsimd.memset(c_cur[:], 0.0)

            # Z stage ring: one tile holds one rc-block (128 rows x 4 gates
            # x 512) of the input projection = 2 timesteps worth.  Split into
            # half-blocks (2 gate-chunks) so the PE filler between a step's
            # recurrence matmuls and its transposes stays within PSUM budget;
            # psum->stage copies go on the idle Scalar engine afterwards.
            def emit_phaseA_half_mms(rc, hb, xth_rc, xtl_rc):
                pss = []
                for n in (2 * hb, 2 * hb + 1):
                    ps = aps.tile([128, 512], f32, tag="aps")
                    for kc in range(5):
                        for pi, (lh, rh) in enumerate((
                                (xth_rc, wxh_t), (xth_rc, wxl_t),
                                (xtl_rc, wxh_t))):
                            nc.tensor.matmul(
                                ps[:], lh[:, kc, :], rh[:, kc, bass.ts(n, 512)],
                                start=(kc == 0 and pi == 0), stop=False,
                            )
                    # += bias (hi+lo fp16 rows against a ones stationary)
                    nc.tensor.matmul(ps[:], ones_t[:], biasw_t[:, n, :],
                                     start=False, stop=True)
                    pss.append((n, ps))
                return pss

            def emit_phaseA_half_copies(stage, pss):
                for n, ps in pss:
                    nc.scalar.copy(stage[:, n, :], ps[:])

            def emit_phaseC_block(rc):
                pl = clps.tile([128, NCC], f32, tag="pl")
                first, last = (0, 0), (3, 2)
                for kc in range(4):
                    hh = HTH[kc // 2][:, kc % 2, bass.ds(BS + rc * 128, 128)]
                    hl = HTL[kc // 2][:, kc % 2, bass.ds(BS + rc * 128, 128)]
                    for pi, (lh, rh) in enumerate((
                            (hh, smwh_t), (hh, smwl_t), (hl, smwh_t))):
                        nc.tensor.matmul(
                            pl[:], lh, rh[:, kc, :],
                            start=((kc, pi) == first), stop=((kc, pi) == last),
                        )
                nc.vector.tensor_tensor(out=logits[:, rc, :], in0=pl[:],
                                        in1=smb_t[:], op=AluOp.add)
                rm1 = pp.tile([128, 16], f32, name=f"rm1_{rc}", tag="rm1")
                nc.vector.tensor_reduce(rm1[:, 0:1], logits[:, rc, :],
                                        axis=mybir.AxisListType.X, op=AluOp.max)
                nc.vector.tensor_scalar(out=logits[:, rc, :], in0=logits[:, rc, :],
                                        scalar1=rm1[:, 0:1], scalar2=None,
                                        op0=AluOp.is_equal)
                nc.vector.tensor_tensor(out=logits[:, rc, :], in0=logits[:, rc, :],
                                        in1=revidx_t[:], op=AluOp.mult)
                nc.vector.tensor_reduce(rm1[:, 1:2], logits[:, rc, :],
                                        axis=mybir.AxisListType.X, op=AluOp.max)
                nc.vector.tensor_scalar(out=logits[:, rc, :], in0=revidx_t[:],
                                        scalar1=rm1[:, 1:2], scalar2=None,
                                        op0=AluOp.is_equal)
                nc.sync.dma_start(onehot_d[bass.ts(rc, 128), :], logits[:, rc, :])

            stages = {}
            h2 = None

            def new_ablock(rc):
                xth_rc = axt.tile([128, 5, 128], f16, tag="xth")
                nc.sync.dma_start(xth_rc[:], xth_r[:, :, rc * 128:(rc + 1) * 128])
                xtl_rc = axt.tile([128, 5, 128], f16, tag="xtl")
                nc.sync.dma_start(xtl_rc[:], xtl_r[:, :, rc * 128:(rc + 1) * 128])
                stage = ap_.tile([128, 4, 512], f32, tag="stage")
                return [rc, stage, xth_rc, xtl_rc]

            # prelude: first two rc blocks fully materialized
            for rc in (0, 1):
                blk = new_ablock(rc)
                for hb in range(2):
                    pss = emit_phaseA_half_mms(rc, hb, blk[2], blk[3])
                    emit_phaseA_half_copies(blk[1], pss)
                stages[rc] = blk[1]
            cur_blk = new_ablock(2)
            nxt_blk = new_ablock(3)
            next_half = 0
            pending = None  # (blk, next_half_flag, pss) copies deferred to next step

            C_SCHED = {28: [0, 1, 2, 3], 29: [4, 5, 6, 7],
                       30: [8, 9, 10, 14], 31: [11, 12, 13]}

            for t in range(T):
                # deferred phase-A psum->stage copies from the previous step's
                # filler run early in the Scalar stream, freeing psum before
                # this step's filler matmuls need it
                if pending is not None:
                    pblk, phalf, ppss = pending
                    emit_phaseA_half_copies(pblk[1], ppss)
                    if phalf == 1:
                        stages[pblk[0]] = pblk[1]
                        cur_blk = nxt_blk
                        nxt_blk = (new_ablock(pblk[0] + 2)
                                   if pblk[0] + 2 <= 15 else None)
                        next_half = 0
                    else:
                        next_half = 1
                    pending = None

                rc, half = divmod(t, 2)
                zin = stages[rc][bass.ds(half * BS, BS), :, :]  # [64, 4, 512]

                # ---- recurrence matmuls for step t ----
                # gates: 0=i, 1=f (zpa), 2=g, 3=o (zpb)
                zpa = bps.tile([128, 2, 512], f32, tag="zpa")
                zpb = bps.tile([128, 2, 512], f32, tag="zpb")
                for pi, (ka, kb) in enumerate(((0, 1), (2, 3))):
                    hha = HTH[ka // 2][:, ka % 2, bass.ds(t * BS, BS)]
                    hla = HTL[ka // 2][:, ka % 2, bass.ds(t * BS, BS)]
                    hhb = HTH[kb // 2][:, kb % 2, bass.ds(t * BS, BS)]
                    hlb = HTL[kb // 2][:, kb % 2, bass.ds(t * BS, BS)]
                    for n in range(4):
                        ps = zpa if n < 2 else zpb
                        g = n % 2
                        wsl = bass.ts(n, 512)
                        for qi, (la, lb, rh) in enumerate((
                                (hha, hhb, whh_t), (hha, hhb, whl_t),
                                (hla, hlb, whh_t))):
                            st = (pi == 0 and qi == 0)
                            sp = (pi == 1 and qi == 2)
                            nc.tensor.matmul(
                                ps[0:64, g, :], la, rh[:, ka, wsl],
                                start=st, stop=sp, tile_position=(0, 0),
                            )
                            nc.tensor.matmul(
                                ps[64:128, g, :], lb, rh[:, kb, wsl],
                                start=st, stop=sp, tile_position=(0, 64),
                            )

                # ---- PE filler emitted before the pointwise/transposes so the
                # in-order tensor stream works on it while the gate chain runs
                if cur_blk is not None:
                    pending = (cur_blk, next_half,
                               emit_phaseA_half_mms(cur_blk[0], next_half,
                                                    cur_blk[2], cur_blk[3]))
                for crc in C_SCHED.get(t, []):
                    emit_phaseC_block(crc)

                # ---- gate pointwise, h update, transposed fp16 hi/lo state ----
                za = bp.tile([BS, 4, 512], f32, tag="za")
                sif = bp1.tile([BS, 2, 512], f32, tag="sif")
                so = bp1.tile([BS, 512], f32, tag="so")
                m1 = bp1.tile([BS, 512], f32, tag="m1")
                m2 = bp1.tile([BS, 512], f32, tag="m2")
                c_new = bp.tile([BS, RNN], f32, tag="c")
                tc2 = bp1.tile([BS, 512], f32, tag="tc2")
                h2 = bp1.tile([BS, RNN], f32, tag="h2")
                h2h = bp1.tile([BS, RNN], f16, tag="h2h")
                h2l = bp1.tile([BS, RNN], f16, tag="h2l")
                trp = bps.tile([128, 8, 64], f16, tag="zpb")

                for hf in range(2):  # rnn halves: cols hf*256:(hf+1)*256
                    hs = bass.ds(hf * 256, 256)
                    # z' = zp_lo + zin, then += zp_hi  (one PSUM input per op)
                    nc.vector.tensor_tensor(out=za[:, 0:2, hs], in0=zpa[0:64, :, hs],
                                            in1=zin[:, 0:2, hs], op=AluOp.add)
                    nc.vector.tensor_tensor(out=za[:, 0:2, hs], in0=za[:, 0:2, hs],
                                            in1=zpa[64:128, :, hs], op=AluOp.add)
                    nc.vector.tensor_tensor(out=za[:, 2:4, hs], in0=zpb[0:64, :, hs],
                                            in1=zin[:, 2:4, hs], op=AluOp.add)
                    nc.vector.tensor_tensor(out=za[:, 2:4, hs], in0=za[:, 2:4, hs],
                                            in1=zpb[64:128, :, hs], op=AluOp.add)
                    # za = tanh(z'); i/f/o columns pre-scaled by 0.5 on host
                    nc.scalar.activation(za[:, :, hs], za[:, :, hs], Act.Tanh)
                    nc.vector.tensor_scalar(out=sif[:, :, hs], in0=za[:, 0:2, hs],
                                            scalar1=0.5, scalar2=0.5,
                                            op0=AluOp.mult, op1=AluOp.add)
                    nc.gpsimd.tensor_scalar(out=so[:, hs], in0=za[:, 3, hs],
                                            scalar1=0.5, scalar2=0.5,
                                            op0=AluOp.mult, op1=AluOp.add)
                    nc.gpsimd.tensor_tensor(out=m1[:, hs], in0=sif[:, 1, hs],
                                            in1=c_cur[:, hs], op=AluOp.mult)
                    nc.vector.tensor_tensor(out=m2[:, hs], in0=sif[:, 0, hs],
                                            in1=za[:, 2, hs], op=AluOp.mult)
                    nc.vector.tensor_tensor(out=c_new[:, hs], in0=m1[:, hs],
                                            in1=m2[:, hs], op=AluOp.add)
                    nc.scalar.activation(tc2[:, hs], c_new[:, hs], Act.Tanh)
                    nc.vector.tensor_tensor(out=h2[:, hs], in0=so[:, hs],
                                            in1=tc2[:, hs], op=AluOp.mult)
                    # fp16 hi/lo split of h2 for the next step's matmuls
                    nc.vector.tensor_copy(h2h[:, hs], h2[:, hs])
                    nc.vector.tensor_tensor(out=h2l[:, hs], in0=h2[:, hs],
                                            in1=h2h[:, hs], op=AluOp.subtract)
                    for kc in (2 * hf, 2 * hf + 1):
                        nc.tensor.transpose(trp[:, kc, :], h2h[:, bass.ts(kc, 128)],
                                            ident[0:64, 0:64])
                        nc.tensor.transpose(trp[:, 4 + kc, :],
                                            h2l[:, bass.ts(kc, 128)],
                                            ident[0:64, 0:64])
                    nc.vector.tensor_copy(
                        HTH[hf][:, :, bass.ds((t + 1) * BS, BS)],
                        trp[:, 2 * hf:2 * hf + 2, :])
                    nc.vector.tensor_copy(
                        HTL[hf][:, :, bass.ds((t + 1) * BS, BS)],
                        trp[:, 4 + 2 * hf:4 + 2 * hf + 2, :])
                c_cur = c_new

            if pending is not None:
                emit_phaseA_half_copies(pending[0][1], pending[2])
            emit_phaseC_block(15)
            nc.sync.dma_start(h_d[:], h2[:])
            nc.sync.dma_start(c_d[:], c_cur[:])

    nc.finalize()
    return nc


def _get_program():
    global _PROGRAM
    if _PROGRAM is None:
        _PROGRAM = _build_program()
    return _PROGRAM


def _prep_inputs(f_pool, ground_truth, kernel, rec_kernel, bias, softmax_w,
                 softmax_b):
    def split16(x):
        hi = x.astype(np.float16)
        lo = (x - hi.astype(np.float32)).astype(np.float16)
        return np.ascontiguousarray(hi), np.ascontiguousarray(lo)

    # fold the tanh half-angle pre-scale of gates i, f, o into the weights
    col_scale = np.ones((G4,), np.float32)
    col_scale[0 * RNN:2 * RNN] = 0.5     # i, f
    col_scale[3 * RNN:4 * RNN] = 0.5     # o
    wxh, wxl = split16(kernel * col_scale[None, :])
    whh, whl = split16(rec_kernel * col_scale[None, :])
    bias_s = (bias * col_scale).astype(np.float32)
    bh, bl = split16(bias_s[None, :])
    biasw = np.ascontiguousarray(np.concatenate([bh, bl], axis=0))
    smbb = np.ascontiguousarray(np.tile(softmax_b[None, :].astype(np.float32),
                                        (128, 1)))
    revidx = np.ascontiguousarray(
        np.tile((NCC - np.arange(NCC, dtype=np.float32))[None, :], (128, 1)))
    smwh, smwl = split16(softmax_w.astype(np.float32))

    in_maps = []
    for c in range(NCORES):
        fp = f_pool[c * BS:(c + 1) * BS]          # [64, 32, 512]
        gt = ground_truth[c * BS:(c + 1) * BS]    # [64, 32, 128]
        prev = np.zeros_like(gt)
        prev[:, 1:] = gt[:, :-1]
        fpT = np.ascontiguousarray(fp).transpose(2, 1, 0).reshape(DEPTH, ROWS)
        prT = np.ascontiguousarray(prev).transpose(2, 1, 0).reshape(NCC, ROWS)
        xt = np.concatenate([fpT, prT], axis=0)
        xth, xtl = split16(xt)
        in_maps.append({
            "xth": xth, "xtl": xtl, "wxh": wxh, "wxl": wxl,
            "whh": whh, "whl": whl, "biasw": biasw,
            "smwh": smwh, "smwl": smwl, "smbb": smbb, "revidx": revidx,
        })
    return in_maps


def _install_ntff_shim():
    """Register the axon NTFF profiling hook the image's antenv lacks."""
    import contextlib, ctypes, sys, types
    try:
        import antenv
    except ImportError:
        return
    if getattr(antenv, "axon_hooks", None) is not None:
        return
    state = {}
    mod = types.ModuleType("antenv.axon_hooks")
    mod.set_axon_ntff_profile_hook = lambda h: state.update(h=h)
    mod.get_axon_ntff_profile_hook = lambda: state.get("h")
    sys.modules["antenv.axon_hooks"] = mod
    antenv.axon_hooks = mod
    try:
        lib = ctypes.CDLL("/opt/axon/libaxon_pjrt.so")
    except OSError:
        return
    if not hasattr(lib, "axon_start_nrt_profile"):
        return
    lib.axon_start_nrt_profile.argtypes = [ctypes.POINTER(ctypes.c_int64),
                                           ctypes.c_size_t]
    lib.axon_start_nrt_profile.restype = ctypes.c_int64
    lib.axon_stop_nrt_profile.argtypes = [ctypes.c_char_p]
    lib.axon_stop_nrt_profile.restype = ctypes.c_int64

    @contextlib.contextmanager
    def _hook(output_dir, device_ids):
        import jax
        jax.devices()
        if device_ids:
            ids = (ctypes.c_int64 * len(device_ids))(*device_ids)
            rc = lib.axon_start_nrt_profile(ids, len(device_ids))
        else:
            rc = lib.axon_start_nrt_profile(None, 0)
        if rc != 0:
            raise RuntimeError(f"axon_start_nrt_profile rc={rc}")
        try:
            yield
        finally:
            n = lib.axon_stop_nrt_profile(str(output_dir).encode())
            if n < 0:
                raise RuntimeError(f"axon_stop_nrt_profile rc={n}")

    mod.set_axon_ntff_profile_hook(_hook)


def _run(in_maps, trace=False):
    from concourse.bass_utils import run_bass_kernel_spmd
    if trace:
        _install_ntff_shim()
    nc = _get_program()
    return run_bass_kernel_spmd(nc, in_maps, list(range(NCORES)), trace=trace)


def kernel(f_pool, ground_truth, kernel, rec_kernel, bias, softmax_w,
           softmax_b):
    f_pool = np.asarray(f_pool, np.float32)
    ground_truth = np.asarray(ground_truth, np.float32)
    in_maps = _prep_inputs(f_pool, ground_truth, np.asarray(kernel, np.float32),
                           np.asarray(rec_kernel, np.float32),
                           np.asarray(bias, np.float32),
                           np.asarray(softmax_w, np.float32),
                           np.asarray(softmax_b, np.float32))
    trace = bool(int(os.environ.get("KERNEL_TRACE", "0")))
    res = _run(in_maps, trace=trace)
    if trace and res.exec_time_ns is not None:
        print(f"HW exec time: {res.exec_time_ns} ns")

    seq = np.empty((B, T, NCC), np.float32)
    h = np.empty((B, RNN), np.float32)
    c = np.empty((B, RNN), np.float32)
    for ci in range(NCORES):
        r = res.results[ci]
        seq[ci * BS:(ci + 1) * BS] = (
            r["onehot"].reshape(T, BS, NCC).transpose(1, 0, 2))
        h[ci * BS:(ci + 1) * BS] = r["h_out"]
        c[ci * BS:(ci + 1) * BS] = r["c_out"]
    return (seq, h, c)


# revision 31
# speedup vs baseline: 1.0266x; 1.0238x over previous
"""ChaRNN LSTM (teacher forcing) Trainium2 Bass kernel.

Structure (data-parallel over batch, 64 rows/core on 8 cores):
  - Teacher forcing means the LSTM input at step t is [f_pool[:,t,:], gt[:,t-1,:]]
    which is fully known ahead of time, so the input projection X @ W_x for all
    32 steps is a big batched matmul (phase A).  Only h @ W_h is sequential
    (phase B).  The argmax/one-hot head is deferred and batched (phase C).
  - Matmuls use fp16 hi/lo 3-pass splits (3 cyc/row total, fp32-grade\n    accuracy, validated 0 argmax flips) - single-pass bf16/fp16 flips
    argmaxes (one-hot output is graded, top-2 logit gaps go down to 6.6e-6).
  - Gate activations use tanh only (4 ULP) via sigmoid(x) = 0.5*tanh(x/2)+0.5;
    the 0.5 pre-scale of the i/f/o gate columns is folded into the weights on
    the host, so one ACT pass computes tanh over the whole 2048-wide gate row.
  - Phase A is emitted interleaved with the recurrence so the tensor engine
    fills the pointwise-tail gaps of each step with input-projection matmuls.
  - Recurrence matmuls pack pairs of K-chunks into PE column halves
    (tile_position (0,0)/(0,64)) since batch=64 only fills half the array;
    measured 1.84x over the unpacked form.
"""

import os
import numpy as np

B, T, DEPTH = 512, 32, 512
RNN, NCC = 512, 128
DIN = DEPTH + NCC            # 640
G4 = 4 * RNN                 # 2048
NCORES = 8
BS = B // NCORES             # 64 batch rows per core
ROWS = T * BS                # 2048 (t-major: r = t*BS + b)

_PROGRAM = None


def _build_program():
    import concourse.bass as bass
    import concourse.tile as tile
    from concourse import bacc, mybir
    from concourse.masks import make_identity

    f32 = mybir.dt.float32
    nc = bacc.Bacc(None)

    f16 = mybir.dt.float16

    xth_d = nc.dram_tensor("xth", [DIN, ROWS], f16, kind="ExternalInput")
    xtl_d = nc.dram_tensor("xtl", [DIN, ROWS], f16, kind="ExternalInput")
    wxh_d = nc.dram_tensor("wxh", [DIN, G4], f16, kind="ExternalInput")
    wxl_d = nc.dram_tensor("wxl", [DIN, G4], f16, kind="ExternalInput")
    whh_d = nc.dram_tensor("whh", [RNN, G4], f16, kind="ExternalInput")
    whl_d = nc.dram_tensor("whl", [RNN, G4], f16, kind="ExternalInput")
    biasw_d = nc.dram_tensor("biasw", [2, G4], f16, kind="ExternalInput")
    smwh_d = nc.dram_tensor("smwh", [RNN, NCC], f16, kind="ExternalInput")
    smwl_d = nc.dram_tensor("smwl", [RNN, NCC], f16, kind="ExternalInput")
    smbb_d = nc.dram_tensor("smbb", [128, NCC], f32, kind="ExternalInput")
    revidx_d = nc.dram_tensor("revidx", [128, NCC], f32, kind="ExternalInput")

    onehot_d = nc.dram_tensor("onehot", [ROWS, NCC], f32, kind="ExternalOutput")
    h_d = nc.dram_tensor("h_out", [BS, RNN], f32, kind="ExternalOutput")
    c_d = nc.dram_tensor("c_out", [BS, RNN], f32, kind="ExternalOutput")

    xth_r = xth_d.rearrange("(kc p) r -> p kc r", p=128)
    xtl_r = xtl_d.rearrange("(kc p) r -> p kc r", p=128)
    wxh_r = wxh_d.rearrange("(kc p) g -> p kc g", p=128)
    wxl_r = wxl_d.rearrange("(kc p) g -> p kc g", p=128)
    whh_r = whh_d.rearrange("(kc p) g -> p kc g", p=128)
    whl_r = whl_d.rearrange("(kc p) g -> p kc g", p=128)
    smwh_r = smwh_d.rearrange("(kc p) n -> p kc n", p=128)
    smwl_r = smwl_d.rearrange("(kc p) n -> p kc n", p=128)

    AluOp = mybir.AluOpType
    Act = mybir.ActivationFunctionType

    with tile.TileContext(nc) as tc:
        with (
            tc.tile_pool(name="persist", bufs=1) as pp,
            tc.tile_pool(name="weights", bufs=1) as wp,
            tc.tile_pool(name="astage", bufs=2) as ap_,
            tc.tile_pool(name="axt", bufs=2) as axt,
            tc.tile_pool(name="bwork", bufs=2) as bp,
            tc.tile_pool(name="bwork1", bufs=1) as bp1,
            tc.tile_pool(name="aps", bufs=3, space="PSUM") as aps,
            tc.tile_pool(name="bps", bufs=1, space="PSUM") as bps,
            tc.tile_pool(name="clps", bufs=1, space="PSUM") as clps,
        ):
            ident = pp.tile([128, 128], f16)
            make_identity(nc, ident[:])
            # h state, fp16 hi/lo pair, transposed; split by rnn half so the
            # next step's first matmul pair only waits on half 0.
            # HTxx[:, kc, s*64+b] = h_(s-1)[b, ...]; block s=0 is h0 = 0.
            HTH = (pp.tile([128, 2, (T + 1) * BS], f16, name="hth0"),
                   pp.tile([128, 2, (T + 1) * BS], f16, name="hth1"))
            HTL = (pp.tile([128, 2, (T + 1) * BS], f16, name="htl0"),
                   pp.tile([128, 2, (T + 1) * BS], f16, name="htl1"))
            for tl in (*HTH, *HTL):
                nc.gpsimd.memset(tl[:, :, 0:BS], 0.0)

            wxh_t = wp.tile([128, 5, G4], f16)
            wxl_t = wp.tile([128, 5, G4], f16)
            whh_t = wp.tile([128, 4, G4], f16)
            whl_t = wp.tile([128, 4, G4], f16)
            for kc in range(5):
                nc.sync.dma_start(wxh_t[:, kc, :], wxh_r[:, kc, :])
                nc.sync.dma_start(wxl_t[:, kc, :], wxl_r[:, kc, :])
            for kc in range(4):
                nc.sync.dma_start(whh_t[:, kc, :], whh_r[:, kc, :])
                nc.sync.dma_start(whl_t[:, kc, :], whl_r[:, kc, :])
            biasw_t = wp.tile([2, 4, 512], f16)
            nc.sync.dma_start(biasw_t[:], biasw_d.rearrange("k (n x) -> k n x", n=4))
            ones_t = wp.tile([2, 128], f16)
            nc.gpsimd.memset(ones_t[:], 1.0)
            smwh_t = wp.tile([128, 4, NCC], f16)
            nc.sync.dma_start(smwh_t[:], smwh_r[:])
            smwl_t = wp.tile([128, 4, NCC], f16)
            nc.sync.dma_start(smwl_t[:], smwl_r[:])
            smb_t = wp.tile([128, NCC], f32)
            nc.sync.dma_start(smb_t[:], smbb_d[:])
            revidx_t = wp.tile([128, NCC], f32)
            nc.sync.dma_start(revidx_t[:], revidx_d[:])
            logits = pp.tile([128, 16, NCC], f32)

            c_cur = bp.tile([BS, RNN], f32, tag="c")
            nc.gpsimd.memset(c_cur[:], 0.0)

            # Z stage ring: one tile holds one rc-block (128 rows x 4 gates
            # x 512) of the input projection = 2 timesteps worth.  Split into
            # half-blocks (2 gate-chunks) so the PE filler between a step's
            # recurrence matmuls and its transposes stays within PSUM budget;
            # psum->stage copies go on the idle Scalar engine afterwards.
            def emit_phaseA_half_mms(rc, hb, xth_rc, xtl_rc):
                pss = []
                for n in (2 * hb, 2 * hb + 1):
                    ps = aps.tile([128, 512], f32, tag="aps")
                    for kc in range(5):
                        for pi, (lh, rh) in enumerate((
                                (xth_rc, wxh_t), (xth_rc, wxl_t),
                                (xtl_rc, wxh_t))):
                            nc.tensor.matmul(
                                ps[:], lh[:, kc, :], rh[:, kc, bass.ts(n, 512)],
                                start=(kc == 0 and pi == 0), stop=False,
                            )
                    # += bias (hi+lo fp16 rows against a ones stationary)
                    nc.tensor.matmul(ps[:], ones_t[:], biasw_t[:, n, :],
                                     start=False, stop=True)
                    pss.append((n, ps))
                return pss

            def emit_phaseA_half_copies(stage, pss):
                for n, ps in pss:
                    nc.scalar.copy(stage[:, n, :], ps[:])

            def emit_phaseC_block(rc):
                pl = clps.tile([128, NCC], f32, tag="pl")
                first, last = (0, 0), (3, 2)
                for kc in range(4):
                    hh = HTH[kc // 2][:, kc % 2, bass.ds(BS + rc * 128, 128)]
                    hl = HTL[kc // 2][:, kc % 2, bass.ds(BS + rc * 128, 128)]
                    for pi, (lh, rh) in enumerate((
                            (hh, smwh_t), (hh, smwl_t), (hl, smwh_t))):
                        nc.tensor.matmul(
                            pl[:], lh, rh[:, kc, :],
                            start=((kc, pi) == first), stop=((kc, pi) == last),
                        )
                nc.vector.tensor_tensor(out=logits[:, rc, :], in0=pl[:],
                                        in1=smb_t[:], op=AluOp.add)
                rm1 = pp.tile([128, 16], f32, name=f"rm1_{rc}", tag="rm1")
                nc.vector.tensor_reduce(rm1[:, 0:1], logits[:, rc, :],
                                        axis=mybir.AxisListType.X, op=AluOp.max)
                nc.vector.tensor_scalar(out=logits[:, rc, :], in0=logits[:, rc, :],
                                        scalar1=rm1[:, 0:1], scalar2=None,
                                        op0=AluOp.is_equal)
                nc.vector.tensor_tensor(out=logits[:, rc, :], in0=logits[:, rc, :],
                                        in1=revidx_t[:], op=AluOp.mult)
                nc.vector.tensor_reduce(rm1[:, 1:2], logits[:, rc, :],
                                        axis=mybir.AxisListType.X, op=AluOp.max)
                nc.vector.tensor_scalar(out=logits[:, rc, :], in0=revidx_t[:],
                                        scalar1=rm1[:, 1:2], scalar2=None,
                                        op0=AluOp.is_equal)
                nc.sync.dma_start(onehot_d[bass.ts(rc, 128), :], logits[:, rc, :])

            stages = {}
            h2 = None

            def new_ablock(rc):
                xth_rc = axt.tile([128, 5, 128], f16, tag="xth")
                nc.sync.dma_start(xth_rc[:], xth_r[:, :, rc * 128:(rc + 1) * 128])
                xtl_rc = axt.tile([128, 5, 128], f16, tag="xtl")
                nc.sync.dma_start(xtl_rc[:], xtl_r[:, :, rc * 128:(rc + 1) * 128])
                stage = ap_.tile([128, 4, 512], f32, tag="stage")
                return [rc, stage, xth_rc, xtl_rc]

            # prelude: first two rc blocks fully materialized
            for rc in (0, 1):
                blk = new_ablock(rc)
                for hb in range(2):
                    pss = emit_phaseA_half_mms(rc, hb, blk[2], blk[3])
                    emit_phaseA_half_copies(blk[1], pss)
                stages[rc] = blk[1]
            cur_blk = new_ablock(2)
            nxt_blk = new_ablock(3)
            next_half = 0
            pending = None  # (blk, next_half_flag, pss) copies deferred to next step

            C_SCHED = {28: [0, 1, 2, 3], 29: [4, 5, 6, 7],
                       30: [8, 9, 10, 14], 31: [11, 12, 13]}

            for t in range(T):
                # deferred phase-A psum->stage copies from the previous step's
                # filler run early in the Scalar stream, freeing psum before
                # this step's filler matmuls need it
                if pending is not None:
                    pblk, phalf, ppss = pending
                    emit_phaseA_half_copies(pblk[1], ppss)
                    if phalf == 1:
                        stages[pblk[0]] = pblk[1]
                        cur_blk = nxt_blk
                        nxt_blk = (new_ablock(pblk[0] + 2)
                                   if pblk[0] + 2 <= 15 else None)
                        next_half = 0
                    else:
                        next_half = 1
                    pending = None

                rc, half = divmod(t, 2)
                zin = stages[rc][bass.ds(half * BS, BS), :, :]  # [64, 4, 512]

                # ---- recurrence matmuls for step t ----
                # gates: 0=i, 1=f (zpa), 2=g, 3=o (zpb)
                zpa = bps.tile([128, 2, 512], f32, tag="zpa")
                zpb = bps.tile([128, 2, 512], f32, tag="zpb")
                for pi, (ka, kb) in enumerate(((0, 1), (2, 3))):
                    hha = HTH[ka // 2][:, ka % 2, bass.ds(t * BS, BS)]
                    hla = HTL[ka // 2][:, ka % 2, bass.ds(t * BS, BS)]
                    hhb = HTH[kb // 2][:, kb % 2, bass.ds(t * BS, BS)]
                    hlb = HTL[kb // 2][:, kb % 2, bass.ds(t * BS, BS)]
                    for n in range(4):
                        ps = zpa if n < 2 else zpb
                        g = n % 2
                        wsl = bass.ts(n, 512)
                        for qi, (la, lb, rh) in enumerate((
                                (hha, hhb, whh_t), (hha, hhb, whl_t),
                                (hla, hlb, whh_t))):
                            st = (pi == 0 and qi == 0)
                            sp = (pi == 1 and qi == 2)
                            nc.tensor.matmul(
                                ps[0:64, g, :], la, rh[:, ka, wsl],
                                start=st, stop=sp, tile_position=(0, 0),
                            )
                            nc.tensor.matmul(
                                ps[64:128, g, :], lb, rh[:, kb, wsl],
                                start=st, stop=sp, tile_position=(0, 64),
                            )

                # ---- PE filler emitted before the pointwise/transposes so the
                # in-order tensor stream works on it while the gate chain runs
                if cur_blk is not None and t >= 2:
                    pending = (cur_blk, next_half,
                               emit_phaseA_half_mms(cur_blk[0], next_half,
                                                    cur_blk[2], cur_blk[3]))
                for crc in C_SCHED.get(t, []):
                    emit_phaseC_block(crc)

                # ---- gate pointwise, h update, transposed fp16 hi/lo state ----
                za = bp.tile([BS, 4, 512], f32, tag="za")
                sif = bp1.tile([BS, 2, 512], f32, tag="sif")
                so = bp1.tile([BS, 512], f32, tag="so")
                m1 = bp1.tile([BS, 512], f32, tag="m1")
                m2 = bp1.tile([BS, 512], f32, tag="m2")
                c_new = bp.tile([BS, RNN], f32, tag="c")
                tc2 = bp1.tile([BS, 512], f32, tag="tc2")
                h2 = bp1.tile([BS, RNN], f32, tag="h2")
                h2h = bp1.tile([BS, RNN], f16, tag="h2h")
                h2l = bp1.tile([BS, RNN], f16, tag="h2l")
                trp = bps.tile([128, 8, 64], f16, tag="zpb")

                for hf in range(2):  # rnn halves: cols hf*256:(hf+1)*256
                    hs = bass.ds(hf * 256, 256)
                    # z' = zp_lo + zin, then += zp_hi  (one PSUM input per op)
                    nc.vector.tensor_tensor(out=za[:, 0:2, hs], in0=zpa[0:64, :, hs],
                                            in1=zin[:, 0:2, hs], op=AluOp.add)
                    nc.vector.tensor_tensor(out=za[:, 0:2, hs], in0=za[:, 0:2, hs],
                                            in1=zpa[64:128, :, hs], op=AluOp.add)
                    nc.vector.tensor_tensor(out=za[:, 2:4, hs], in0=zpb[0:64, :, hs],
                                            in1=zin[:, 2:4, hs], op=AluOp.add)
                    nc.vector.tensor_tensor(out=za[:, 2:4, hs], in0=za[:, 2:4, hs],
                                            in1=zpb[64:128, :, hs], op=AluOp.add)
                    # za = tanh(z'); i/f/o columns pre-scaled by 0.5 on host
                    nc.scalar.activation(za[:, :, hs], za[:, :, hs], Act.Tanh)
                    nc.vector.tensor_scalar(out=sif[:, :, hs], in0=za[:, 0:2, hs],
                                            scalar1=0.5, scalar2=0.5,
                                            op0=AluOp.mult, op1=AluOp.add)
                    nc.gpsimd.tensor_scalar(out=so[:, hs], in0=za[:, 3, hs],
                                            scalar1=0.5, scalar2=0.5,
                                            op0=AluOp.mult, op1=AluOp.add)
                    nc.gpsimd.tensor_tensor(out=m1[:, hs], in0=sif[:, 1, hs],
                                            in1=c_cur[:, hs], op=AluOp.mult)
                    nc.vector.tensor_tensor(out=m2[:, hs], in0=sif[:, 0, hs],
                                            in1=za[:, 2, hs], op=AluOp.mult)
                    nc.vector.tensor_tensor(out=c_new[:, hs], in0=m1[:, hs],
                                            in1=m2[:, hs], op=AluOp.add)
                    nc.scalar.activation(tc2[:, hs], c_new[:, hs], Act.Tanh)
                    nc.vector.tensor_tensor(out=h2[:, hs], in0=so[:, hs],
                                            in1=tc2[:, hs], op=AluOp.mult)
                    # fp16 hi/lo split of h2 for the next step's matmuls
                    nc.vector.tensor_copy(h2h[:, hs], h2[:, hs])
                    nc.vector.tensor_tensor(out=h2l[:, hs], in0=h2[:, hs],
                                            in1=h2h[:, hs], op=AluOp.subtract)
                    for kc in (2 * hf, 2 * hf + 1):
                        nc.tensor.transpose(trp[:, kc, :], h2h[:, bass.ts(kc, 128)],
                                            ident[0:64, 0:64])
                        nc.tensor.transpose(trp[:, 4 + kc, :],
                                            h2l[:, bass.ts(kc, 128)],
                                            ident[0:64, 0:64])
                    nc.vector.tensor_copy(
                        HTH[hf][:, :, bass.ds((t + 1) * BS, BS)],
                        trp[:, 2 * hf:2 * hf + 2, :])
                    nc.vector.tensor_copy(
                        HTL[hf][:, :, bass.ds((t + 1) * BS, BS)],
                        trp[:, 4 + 2 * hf:4 + 2 * hf + 2, :])
                c_cur = c_new

            if pending is not None:
                emit_phaseA_half_copies(pending[0][1], pending[2])
            emit_phaseC_block(15)
            nc.sync.dma_start(h_d[:], h2[:])
            nc.sync.dma_start(c_d[:], c_cur[:])

    nc.finalize()
    return nc


def _get_program():
    global _PROGRAM
    if _PROGRAM is None:
        _PROGRAM = _build_program()
    return _PROGRAM


def _prep_inputs(f_pool, ground_truth, kernel, rec_kernel, bias, softmax_w,
                 softmax_b):
    def split16(x):
        hi = x.astype(np.float16)
        lo = (x - hi.astype(np.float32)).astype(np.float16)
        return np.ascontiguousarray(hi), np.ascontiguousarray(lo)

    # fold the tanh half-angle pre-scale of gates i, f, o into the weights
    col_scale = np.ones((G4,), np.float32)
    col_scale[0 * RNN:2 * RNN] = 0.5     # i, f
    col_scale[3 * RNN:4 * RNN] = 0.5     # o
    wxh, wxl = split16(kernel * col_scale[None, :])
    whh, whl = split16(rec_kernel * col_scale[None, :])
    bias_s = (bias * col_scale).astype(np.float32)
    bh, bl = split16(bias_s[None, :])
    biasw = np.ascontiguousarray(np.concatenate([bh, bl], axis=0))
    smbb = np.ascontiguousarray(np.tile(softmax_b[None, :].astype(np.float32),
                                        (128, 1)))
    revidx = np.ascontiguousarray(
        np.tile((NCC - np.arange(NCC, dtype=np.float32))[None, :], (128, 1)))
    smwh, smwl = split16(softmax_w.astype(np.float32))

    in_maps = []
    for c in range(NCORES):
        fp = f_pool[c * BS:(c + 1) * BS]          # [64, 32, 512]
        gt = ground_truth[c * BS:(c + 1) * BS]    # [64, 32, 128]
        prev = np.zeros_like(gt)
        prev[:, 1:] = gt[:, :-1]
        fpT = np.ascontiguousarray(fp).transpose(2, 1, 0).reshape(DEPTH, ROWS)
        prT = np.ascontiguousarray(prev).transpose(2, 1, 0).reshape(NCC, ROWS)
        xt = np.concatenate([fpT, prT], axis=0)
        xth, xtl = split16(xt)
        in_maps.append({
            "xth": xth, "xtl": xtl, "wxh": wxh, "wxl": wxl,
            "whh": whh, "whl": whl, "biasw": biasw,
            "smwh": smwh, "smwl": smwl, "smbb": smbb, "revidx": revidx,
        })
    return in_maps


def _install_ntff_shim():
    """Register the axon NTFF profiling hook the image's antenv lacks."""
    import contextlib, ctypes, sys, types
    try:
        import antenv
    except ImportError:
        return
    if getattr(antenv, "axon_hooks", None) is not None:
        return
    state = {}
    mod = types.ModuleType("antenv.axon_hooks")
    mod.set_axon_ntff_profile_hook = lambda h: state.update(h=h)
    mod.get_axon_ntff_profile_hook = lambda: state.get("h")
    sys.modules["antenv.axon_hooks"] = mod
    antenv.axon_hooks = mod
    try:
        lib = ctypes.CDLL("/opt/axon/libaxon_pjrt.so")
    except OSError:
        return
    if not hasattr(lib, "axon_start_nrt_profile"):
        return
    lib.axon_start_nrt_profile.argtypes = [ctypes.POINTER(ctypes.c_int64),
                                           ctypes.c_size_t]
    lib.axon_start_nrt_profile.restype = ctypes.c_int64
    lib.axon_stop_nrt_profile.argtypes = [ctypes.c_char_p]
    lib.axon_stop_nrt_profile.restype = ctypes.c_int64

    @contextlib.contextmanager
    def _hook(output_dir, device_ids):
        import jax
        jax.devices()
        if device_ids:
            ids = (ctypes.c_int64 * len(device_ids))(*device_ids)
            rc = lib.axon_start_nrt_profile(ids, len(device_ids))
        else:
            rc = lib.axon_start_nrt_profile(None, 0)
        if rc != 0:
            raise RuntimeError(f"axon_start_nrt_profile rc={rc}")
        try:
            yield
        finally:
            n = lib.axon_stop_nrt_profile(str(output_dir).encode())
            if n < 0:
                raise RuntimeError(f"axon_stop_nrt_profile rc={n}")

    mod.set_axon_ntff_profile_hook(_hook)


def _run(in_maps, trace=False):
    from concourse.bass_utils import run_bass_kernel_spmd
    if trace:
        _install_ntff_shim()
    nc = _get_program()
    return run_bass_kernel_spmd(nc, in_maps, list(range(NCORES)), trace=trace)


def kernel(f_pool, ground_truth, kernel, rec_kernel, bias, softmax_w,
           softmax_b):
    f_pool = np.asarray(f_pool, np.float32)
    ground_truth = np.asarray(ground_truth, np.float32)
    in_maps = _prep_inputs(f_pool, ground_truth, np.asarray(kernel, np.float32),
                           np.asarray(rec_kernel, np.float32),
                           np.asarray(bias, np.float32),
                           np.asarray(softmax_w, np.float32),
                           np.asarray(softmax_b, np.float32))
    trace = bool(int(os.environ.get("KERNEL_TRACE", "0")))
    res = _run(in_maps, trace=trace)
    if trace and res.exec_time_ns is not None:
        print(f"HW exec time: {res.exec_time_ns} ns")

    seq = np.empty((B, T, NCC), np.float32)
    h = np.empty((B, RNN), np.float32)
    c = np.empty((B, RNN), np.float32)
    for ci in range(NCORES):
        r = res.results[ci]
        seq[ci * BS:(ci + 1) * BS] = (
            r["onehot"].reshape(T, BS, NCC).transpose(1, 0, 2))
        h[ci * BS:(ci + 1) * BS] = r["h_out"]
        c[ci * BS:(ci + 1) * BS] = r["c_out"]
    return (seq, h, c)


# revision 32
# speedup vs baseline: 1.0691x; 1.0414x over previous
"""ChaRNN LSTM (teacher forcing) Trainium2 Bass kernel.

Structure (data-parallel over batch, 64 rows/core on 8 cores):
  - Teacher forcing means the LSTM input at step t is [f_pool[:,t,:], gt[:,t-1,:]]
    which is fully known ahead of time, so the input projection X @ W_x for all
    32 steps is a big batched matmul (phase A).  Only h @ W_h is sequential
    (phase B).  The argmax/one-hot head is deferred and batched (phase C).
  - Matmuls use fp16 hi/lo 3-pass splits (3 cyc/row total, fp32-grade\n    accuracy, validated 0 argmax flips) - single-pass bf16/fp16 flips
    argmaxes (one-hot output is graded, top-2 logit gaps go down to 6.6e-6).
  - Gate activations use tanh only (4 ULP) via sigmoid(x) = 0.5*tanh(x/2)+0.5;
    the 0.5 pre-scale of the i/f/o gate columns is folded into the weights on
    the host, so one ACT pass computes tanh over the whole 2048-wide gate row.
  - Phase A is emitted interleaved with the recurrence so the tensor engine
    fills the pointwise-tail gaps of each step with input-projection matmuls.
  - Recurrence matmuls pack pairs of K-chunks into PE column halves
    (tile_position (0,0)/(0,64)) since batch=64 only fills half the array;
    measured 1.84x over the unpacked form.
"""

import os
import numpy as np

B, T, DEPTH = 512, 32, 512
RNN, NCC = 512, 128
DIN = DEPTH + NCC            # 640
G4 = 4 * RNN                 # 2048
NCORES = 8
BS = B // NCORES             # 64 batch rows per core
ROWS = T * BS                # 2048 (t-major: r = t*BS + b)

_PROGRAM = None


def _build_program():
    import concourse.bass as bass
    import concourse.tile as tile
    from concourse import bacc, mybir
    from concourse.masks import make_identity

    f32 = mybir.dt.float32
    nc = bacc.Bacc(None)

    f16 = mybir.dt.float16

    xth_d = nc.dram_tensor("xth", [DIN, ROWS], f16, kind="ExternalInput")
    xtl_d = nc.dram_tensor("xtl", [DIN, ROWS], f16, kind="ExternalInput")
    wxh_d = nc.dram_tensor("wxh", [DIN, G4], f16, kind="ExternalInput")
    wxl_d = nc.dram_tensor("wxl", [DIN, G4], f16, kind="ExternalInput")
    whh_d = nc.dram_tensor("whh", [RNN, G4], f16, kind="ExternalInput")
    whl_d = nc.dram_tensor("whl", [RNN, G4], f16, kind="ExternalInput")
    biasw_d = nc.dram_tensor("biasw", [2, G4], f16, kind="ExternalInput")
    smwh_d = nc.dram_tensor("smwh", [RNN, NCC], f16, kind="ExternalInput")
    smwl_d = nc.dram_tensor("smwl", [RNN, NCC], f16, kind="ExternalInput")
    smbb_d = nc.dram_tensor("smbb", [128, NCC], f32, kind="ExternalInput")
    revidx_d = nc.dram_tensor("revidx", [128, NCC], f32, kind="ExternalInput")

    onehot_d = nc.dram_tensor("onehot", [ROWS, NCC], f32, kind="ExternalOutput")
    h_d = nc.dram_tensor("h_out", [BS, RNN], f32, kind="ExternalOutput")
    c_d = nc.dram_tensor("c_out", [BS, RNN], f32, kind="ExternalOutput")

    xth_r = xth_d.rearrange("(kc p) r -> p kc r", p=128)
    xtl_r = xtl_d.rearrange("(kc p) r -> p kc r", p=128)
    wxh_r = wxh_d.rearrange("(kc p) g -> p kc g", p=128)
    wxl_r = wxl_d.rearrange("(kc p) g -> p kc g", p=128)
    whh_r = whh_d.rearrange("(kc p) g -> p kc g", p=128)
    whl_r = whl_d.rearrange("(kc p) g -> p kc g", p=128)
    smwh_r = smwh_d.rearrange("(kc p) n -> p kc n", p=128)
    smwl_r = smwl_d.rearrange("(kc p) n -> p kc n", p=128)

    AluOp = mybir.AluOpType
    Act = mybir.ActivationFunctionType

    with tile.TileContext(nc) as tc:
        with (
            tc.tile_pool(name="persist", bufs=1) as pp,
            tc.tile_pool(name="weights", bufs=1) as wp,
            tc.tile_pool(name="astage", bufs=2) as ap_,
            tc.tile_pool(name="axt", bufs=2) as axt,
            tc.tile_pool(name="bwork", bufs=2) as bp,
            tc.tile_pool(name="bwork1", bufs=1) as bp1,
            tc.tile_pool(name="aps", bufs=3, space="PSUM") as aps,
            tc.tile_pool(name="bps", bufs=1, space="PSUM") as bps,
            tc.tile_pool(name="clps", bufs=1, space="PSUM") as clps,
            tc.tile_pool(name="clps", bufs=1, space="PSUM") as clps,
        ):
            ident = pp.tile([128, 128], f32)
            make_identity(nc, ident[:])
            # h state, transposed, fp16 hi/lo packed along the stationary M
            # dim: HTS[hf][:, kc, s, 0:64] = h_hi^T, [..., 64:128] = h_lo^T.
            # One matmul per (kc, W-pass) computes both hi and lo partial
            # products into psum partition halves (summed by the existing
            # halves-add), so W_hi/W_lo each stream only once per step.
            # Block s=0 is h0 = 0.
            HTS = (pp.tile([128, 2, T + 1, 128], f16, name="hts0"),
                   pp.tile([128, 2, T + 1, 128], f16, name="hts1"))
            for tl in HTS:
                nc.gpsimd.memset(tl[:, :, 0, :], 0.0)

            wxh_t = wp.tile([128, 5, G4], f16)
            wxl_t = wp.tile([128, 5, G4], f16)
            whh_t = wp.tile([128, 4, G4], f16)
            whl_t = wp.tile([128, 4, G4], f16)
            for kc in range(5):
                nc.sync.dma_start(wxh_t[:, kc, :], wxh_r[:, kc, :])
                nc.sync.dma_start(wxl_t[:, kc, :], wxl_r[:, kc, :])
            for kc in range(4):
                nc.sync.dma_start(whh_t[:, kc, :], whh_r[:, kc, :])
                nc.sync.dma_start(whl_t[:, kc, :], whl_r[:, kc, :])
            biasw_t = wp.tile([2, 4, 512], f16)
            nc.sync.dma_start(biasw_t[:], biasw_d.rearrange("k (n x) -> k n x", n=4))
            ones_t = wp.tile([2, 128], f16)
            nc.gpsimd.memset(ones_t[:], 1.0)
            smwh_t = wp.tile([128, 4, NCC], f16)
            nc.sync.dma_start(smwh_t[:], smwh_r[:])
            smwl_t = wp.tile([128, 4, NCC], f16)
            nc.sync.dma_start(smwl_t[:], smwl_r[:])
            smb_t = wp.tile([128, NCC], f32)
            nc.sync.dma_start(smb_t[:], smbb_d[:])
            revidx_t = wp.tile([128, NCC], f32)
            nc.sync.dma_start(revidx_t[:], revidx_d[:])
            logits = pp.tile([128, 16, NCC], f32)

            c_cur = bp.tile([BS, RNN], f32, tag="c")
            nc.gpsimd.memset(c_cur[:], 0.0)

            # Z stage ring: one tile holds one rc-block (128 rows x 4 gates
            # x 512) of the input projection = 2 timesteps worth.  Split into
            # half-blocks (2 gate-chunks) so the PE filler between a step's
            # recurrence matmuls and its transposes stays within PSUM budget;
            # psum->stage copies go on the idle Scalar engine afterwards.
            def emit_phaseA_half_mms(rc, hb, xth_rc, xtl_rc):
                pss = []
                for n in (2 * hb, 2 * hb + 1):
                    ps = aps.tile([128, 512], f32, tag="aps")
                    for kc in range(5):
                        for pi, (lh, rh) in enumerate((
                                (xth_rc, wxh_t), (xth_rc, wxl_t),
                                (xtl_rc, wxh_t))):
                            nc.tensor.matmul(
                                ps[:], lh[:, kc, :], rh[:, kc, bass.ts(n, 512)],
                                start=(kc == 0 and pi == 0), stop=False,
                            )
                    # += bias (hi+lo fp16 rows against a ones stationary)
                    nc.tensor.matmul(ps[:], ones_t[:], biasw_t[:, n, :],
                                     start=False, stop=True)
                    pss.append((n, ps))
                return pss

            def emit_phaseA_half_copies(stage, pss):
                for n, ps in pss:
                    nc.scalar.copy(stage[:, n, :], ps[:])

            def emit_phaseC_block(rc):
                # two 64-row half-blocks; stationary is the packed [hi|lo]
                # state block: psum rows 0:64 = hi terms, 64:128 = lo terms
                for sb in range(2):
                    pl = clps.tile([128, NCC], f32, tag="pl")
                    for kc in range(4):
                        hts = HTS[kc // 2][:, kc % 2, 1 + rc * 2 + sb, :]
                        for qi, rh in enumerate((smwh_t, smwl_t)):
                            nc.tensor.matmul(
                                pl[:], hts, rh[:, kc, :],
                                start=(kc == 0 and qi == 0),
                                stop=(kc == 3 and qi == 1),
                            )
                    out_sl = logits[bass.ds(sb * 64, 64), rc, :]
                    nc.vector.tensor_tensor(out=out_sl, in0=pl[0:64, :],
                                            in1=smb_t[0:64, :], op=AluOp.add)
                    nc.vector.tensor_tensor(out=out_sl, in0=out_sl,
                                            in1=pl[64:128, :], op=AluOp.add)
                rm1 = pp.tile([128, 16], f32, name=f"rm1_{rc}", tag="rm1")
                nc.vector.tensor_reduce(rm1[:, 0:1], logits[:, rc, :],
                                        axis=mybir.AxisListType.X, op=AluOp.max)
                nc.vector.tensor_scalar(out=logits[:, rc, :], in0=logits[:, rc, :],
                                        scalar1=rm1[:, 0:1], scalar2=None,
                                        op0=AluOp.is_equal)
                nc.vector.tensor_tensor(out=logits[:, rc, :], in0=logits[:, rc, :],
                                        in1=revidx_t[:], op=AluOp.mult)
                nc.vector.tensor_reduce(rm1[:, 1:2], logits[:, rc, :],
                                        axis=mybir.AxisListType.X, op=AluOp.max)
                nc.vector.tensor_scalar(out=logits[:, rc, :], in0=revidx_t[:],
                                        scalar1=rm1[:, 1:2], scalar2=None,
                                        op0=AluOp.is_equal)
                nc.sync.dma_start(onehot_d[bass.ts(rc, 128), :], logits[:, rc, :])

            stages = {}
            h2 = None

            def new_ablock(rc):
                xth_rc = axt.tile([128, 5, 128], f16, tag="xth")
                nc.sync.dma_start(xth_rc[:], xth_r[:, :, rc * 128:(rc + 1) * 128])
                xtl_rc = axt.tile([128, 5, 128], f16, tag="xtl")
                nc.sync.dma_start(xtl_rc[:], xtl_r[:, :, rc * 128:(rc + 1) * 128])
                stage = ap_.tile([128, 4, 512], f32, tag="stage")
                return [rc, stage, xth_rc, xtl_rc]

            # prelude: first two rc blocks fully materialized
            for rc in (0, 1):
                blk = new_ablock(rc)
                for hb in range(2):
                    pss = emit_phaseA_half_mms(rc, hb, blk[2], blk[3])
                    emit_phaseA_half_copies(blk[1], pss)
                stages[rc] = blk[1]
            cur_blk = new_ablock(2)
            nxt_blk = new_ablock(3)
            next_half = 0
            pending = None  # (blk, next_half_flag, pss) copies deferred to next step

            C_SCHED = {28: [0, 1, 2, 3], 29: [4, 5, 6, 7],
                       30: [8, 9, 10, 14], 31: [11, 12, 13]}

            for t in range(T):
                # deferred phase-A psum->stage copies from the previous step's
                # filler run early in the Scalar stream, freeing psum before
                # this step's filler matmuls need it
                if pending is not None:
                    pblk, phalf, ppss = pending
                    emit_phaseA_half_copies(pblk[1], ppss)
                    if phalf == 1:
                        stages[pblk[0]] = pblk[1]
                        cur_blk = nxt_blk
                        nxt_blk = (new_ablock(pblk[0] + 2)
                                   if pblk[0] + 2 <= 15 else None)
                        next_half = 0
                    else:
                        next_half = 1
                    pending = None

                rc, half = divmod(t, 2)
                zin = stages[rc][bass.ds(half * BS, BS), :, :]  # [64, 4, 512]

                # ---- recurrence matmuls for step t ----
                # gates: 0=i, 1=f (zpa), 2=g, 3=o (zpb)
                zpa = bps.tile([128, 2, 512], f32, tag="zpa")
                zpb = bps.tile([128, 2, 512], f32, tag="zpb")
                for gp, ps in ((0, zpa), (1, zpb)):
                    for kc in range(4):
                        hts = HTS[kc // 2][:, kc % 2, t, :]
                        for qi, rh in enumerate((whh_t, whl_t)):
                            for ni in range(2):
                                nc.tensor.matmul(
                                    ps[:, ni, :], hts,
                                    rh[:, kc, bass.ts(2 * gp + ni, 512)],
                                    start=(kc == 0 and qi == 0),
                                    stop=(kc == 3 and qi == 1),
                                )

                # ---- PE filler emitted before the pointwise/transposes so the
                # in-order tensor stream works on it while the gate chain runs
                if cur_blk is not None:
                    pending = (cur_blk, next_half,
                               emit_phaseA_half_mms(cur_blk[0], next_half,
                                                    cur_blk[2], cur_blk[3]))
                for crc in C_SCHED.get(t, []):
                    emit_phaseC_block(crc)

                # ---- gate pointwise, h update, transposed fp16 hi/lo state ----
                za = bp.tile([BS, 4, 512], f32, tag="za")
                sif = bp1.tile([BS, 2, 512], f32, tag="sif")
                so = bp1.tile([BS, 512], f32, tag="so")
                m1 = bp1.tile([BS, 512], f32, tag="m1")
                m2 = bp1.tile([BS, 512], f32, tag="m2")
                c_new = bp.tile([BS, RNN], f32, tag="c")
                tc2 = bp1.tile([BS, 512], f32, tag="tc2")
                h2 = bp1.tile([BS, RNN], f32, tag="h2")
                trp = bps.tile([128, 4, 64], f32, tag="zpb")

                for hf in range(2):  # rnn halves: cols hf*256:(hf+1)*256
                    hs = bass.ds(hf * 256, 256)
                    # z' = zp_lo + zin, then += zp_hi  (one PSUM input per op)
                    nc.vector.tensor_tensor(out=za[:, 0:2, hs], in0=zpa[0:64, :, hs],
                                            in1=zin[:, 0:2, hs], op=AluOp.add)
                    nc.vector.tensor_tensor(out=za[:, 0:2, hs], in0=za[:, 0:2, hs],
                                            in1=zpa[64:128, :, hs], op=AluOp.add)
                    nc.vector.tensor_tensor(out=za[:, 2:4, hs], in0=zpb[0:64, :, hs],
                                            in1=zin[:, 2:4, hs], op=AluOp.add)
                    nc.vector.tensor_tensor(out=za[:, 2:4, hs], in0=za[:, 2:4, hs],
                                            in1=zpb[64:128, :, hs], op=AluOp.add)
                    # za = tanh(z'); i/f/o columns pre-scaled by 0.5 on host
                    nc.scalar.activation(za[:, :, hs], za[:, :, hs], Act.Tanh)
                    nc.vector.tensor_scalar(out=sif[:, :, hs], in0=za[:, 0:2, hs],
                                            scalar1=0.5, scalar2=0.5,
                                            op0=AluOp.mult, op1=AluOp.add)
                    nc.gpsimd.tensor_scalar(out=so[:, hs], in0=za[:, 3, hs],
                                            scalar1=0.5, scalar2=0.5,
                                            op0=AluOp.mult, op1=AluOp.add)
                    nc.gpsimd.tensor_tensor(out=m1[:, hs], in0=sif[:, 1, hs],
                                            in1=c_cur[:, hs], op=AluOp.mult)
                    nc.vector.tensor_tensor(out=m2[:, hs], in0=sif[:, 0, hs],
                                            in1=za[:, 2, hs], op=AluOp.mult)
                    nc.vector.tensor_tensor(out=c_new[:, hs], in0=m1[:, hs],
                                            in1=m2[:, hs], op=AluOp.add)
                    nc.scalar.activation(tc2[:, hs], c_new[:, hs], Act.Tanh)
                    nc.vector.tensor_tensor(out=h2[:, hs], in0=so[:, hs],
                                            in1=tc2[:, hs], op=AluOp.mult)
                    # transpose h2 (f32), then split fp16 hi/lo in transposed
                    # space straight into the packed stationary tile
                    for kc in (2 * hf, 2 * hf + 1):
                        nc.tensor.transpose(trp[:, kc, :], h2[:, bass.ts(kc, 128)],
                                            ident[0:64, 0:64])
                    nc.vector.tensor_copy(
                        HTS[hf][:, :, t + 1, 0:64],
                        trp[:, 2 * hf:2 * hf + 2, :])
                    nc.vector.tensor_tensor(
                        out=HTS[hf][:, :, t + 1, 64:128],
                        in0=trp[:, 2 * hf:2 * hf + 2, :],
                        in1=HTS[hf][:, :, t + 1, 0:64], op=AluOp.subtract)
                c_cur = c_new

            if pending is not None:
                emit_phaseA_half_copies(pending[0][1], pending[2])
            emit_phaseC_block(15)
            nc.sync.dma_start(h_d[:], h2[:])
            nc.sync.dma_start(c_d[:], c_cur[:])

    nc.finalize()
    return nc


def _get_program():
    global _PROGRAM
    if _PROGRAM is None:
        _PROGRAM = _build_program()
    return _PROGRAM


def _prep_inputs(f_pool, ground_truth, kernel, rec_kernel, bias, softmax_w,
                 softmax_b):
    def split16(x):
        hi = x.astype(np.float16)
        lo = (x - hi.astype(np.float32)).astype(np.float16)
        return np.ascontiguousarray(hi), np.ascontiguousarray(lo)

    # fold the tanh half-angle pre-scale of gates i, f, o into the weights
    col_scale = np.ones((G4,), np.float32)
    col_scale[0 * RNN:2 * RNN] = 0.5     # i, f
    col_scale[3 * RNN:4 * RNN] = 0.5     # o
    wxh, wxl = split16(kernel * col_scale[None, :])
    whh, whl = split16(rec_kernel * col_scale[None, :])
    bias_s = (bias * col_scale).astype(np.float32)
    bh, bl = split16(bias_s[None, :])
    biasw = np.ascontiguousarray(np.concatenate([bh, bl], axis=0))
    smbb = np.ascontiguousarray(np.tile(softmax_b[None, :].astype(np.float32),
                                        (128, 1)))
    revidx = np.ascontiguousarray(
        np.tile((NCC - np.arange(NCC, dtype=np.float32))[None, :], (128, 1)))
    smwh, smwl = split16(softmax_w.astype(np.float32))

    in_maps = []
    for c in range(NCORES):
        fp = f_pool[c * BS:(c + 1) * BS]          # [64, 32, 512]
        gt = ground_truth[c * BS:(c + 1) * BS]    # [64, 32, 128]
        prev = np.zeros_like(gt)
        prev[:, 1:] = gt[:, :-1]
        fpT = np.ascontiguousarray(fp).transpose(2, 1, 0).reshape(DEPTH, ROWS)
        prT = np.ascontiguousarray(prev).transpose(2, 1, 0).reshape(NCC, ROWS)
        xt = np.concatenate([fpT, prT], axis=0)
        xth, xtl = split16(xt)
        in_maps.append({
            "xth": xth, "xtl": xtl, "wxh": wxh, "wxl": wxl,
            "whh": whh, "whl": whl, "biasw": biasw,
            "smwh": smwh, "smwl": smwl, "smbb": smbb, "revidx": revidx,
        })
    return in_maps


def _install_ntff_shim():
    """Register the axon NTFF profiling hook the image's antenv lacks."""
    import contextlib, ctypes, sys, types
    try:
        import antenv
    except ImportError:
        return
    if getattr(antenv, "axon_hooks", None) is not None:
        return
    state = {}
    mod = types.ModuleType("antenv.axon_hooks")
    mod.set_axon_ntff_profile_hook = lambda h: state.update(h=h)
    mod.get_axon_ntff_profile_hook = lambda: state.get("h")
    sys.modules["antenv.axon_hooks"] = mod
    antenv.axon_hooks = mod
    try:
        lib = ctypes.CDLL("/opt/axon/libaxon_pjrt.so")
    except OSError:
        return
    if not hasattr(lib, "axon_start_nrt_profile"):
        return
    lib.axon_start_nrt_profile.argtypes = [ctypes.POINTER(ctypes.c_int64),
                                           ctypes.c_size_t]
    lib.axon_start_nrt_profile.restype = ctypes.c_int64
    lib.axon_stop_nrt_profile.argtypes = [ctypes.c_char_p]
    lib.axon_stop_nrt_profile.restype = ctypes.c_int64

    @contextlib.contextmanager
    def _hook(output_dir, device_ids):
        import jax
        jax.devices()
        if device_ids:
            ids = (ctypes.c_int64 * len(device_ids))(*device_ids)
            rc = lib.axon_start_nrt_profile(ids, len(device_ids))
        else:
            rc = lib.axon_start_nrt_profile(None, 0)
        if rc != 0:
            raise RuntimeError(f"axon_start_nrt_profile rc={rc}")
        try:
            yield
        finally:
            n = lib.axon_stop_nrt_profile(str(output_dir).encode())
            if n < 0:
                raise RuntimeError(f"axon_stop_nrt_profile rc={n}")

    mod.set_axon_ntff_profile_hook(_hook)


def _run(in_maps, trace=False):
    from concourse.bass_utils import run_bass_kernel_spmd
    if trace:
        _install_ntff_shim()
    nc = _get_program()
    return run_bass_kernel_spmd(nc, in_maps, list(range(NCORES)), trace=trace)


def kernel(f_pool, ground_truth, kernel, rec_kernel, bias, softmax_w,
           softmax_b):
    f_pool = np.asarray(f_pool, np.float32)
    ground_truth = np.asarray(ground_truth, np.float32)
    in_maps = _prep_inputs(f_pool, ground_truth, np.asarray(kernel, np.float32),
                           np.asarray(rec_kernel, np.float32),
                           np.asarray(bias, np.float32),
                           np.asarray(softmax_w, np.float32),
                           np.asarray(softmax_b, np.float32))
    trace = bool(int(os.environ.get("KERNEL_TRACE", "0")))
    res = _run(in_maps, trace=trace)
    if trace and res.exec_time_ns is not None:
        print(f"HW exec time: {res.exec_time_ns} ns")

    seq = np.empty((B, T, NCC), np.float32)
    h = np.empty((B, RNN), np.float32)
    c = np.empty((B, RNN), np.float32)
    for ci in range(NCORES):
        r = res.results[ci]
        seq[ci * BS:(ci + 1) * BS] = (
            r["onehot"].reshape(T, BS, NCC).transpose(1, 0, 2))
        h[ci * BS:(ci + 1) * BS] = r["h_out"]
        c[ci * BS:(ci + 1) * BS] = r["c_out"]
    return (seq, h, c)


# revision 34
# speedup vs baseline: 1.0952x; 1.0244x over previous
"""ChaRNN LSTM (teacher forcing) Trainium2 Bass kernel.

Structure (data-parallel over batch, 64 rows/core on 8 cores):
  - Teacher forcing means the LSTM input at step t is [f_pool[:,t,:], gt[:,t-1,:]]
    which is fully known ahead of time, so the input projection X @ W_x for all
    32 steps is a big batched matmul (phase A).  Only h @ W_h is sequential
    (phase B).  The argmax/one-hot head is deferred and batched (phase C).
  - Matmuls use fp16 hi/lo 3-pass splits (3 cyc/row total, fp32-grade\n    accuracy, validated 0 argmax flips) - single-pass bf16/fp16 flips
    argmaxes (one-hot output is graded, top-2 logit gaps go down to 6.6e-6).
  - Gate activations use tanh only (4 ULP) via sigmoid(x) = 0.5*tanh(x/2)+0.5;
    the 0.5 pre-scale of the i/f/o gate columns is folded into the weights on
    the host, so one ACT pass computes tanh over the whole 2048-wide gate row.
  - Phase A is emitted interleaved with the recurrence so the tensor engine
    fills the pointwise-tail gaps of each step with input-projection matmuls.
  - Recurrence matmuls pack pairs of K-chunks into PE column halves
    (tile_position (0,0)/(0,64)) since batch=64 only fills half the array;
    measured 1.84x over the unpacked form.
"""

import os
import numpy as np

B, T, DEPTH = 512, 32, 512
RNN, NCC = 512, 128
DIN = DEPTH + NCC            # 640
G4 = 4 * RNN                 # 2048
NCORES = 8
BS = B // NCORES             # 64 batch rows per core
ROWS = T * BS                # 2048 (t-major: r = t*BS + b)

_PROGRAM = None


def _build_program():
    import concourse.bass as bass
    import concourse.tile as tile
    from concourse import bacc, mybir
    from concourse.masks import make_identity

    f32 = mybir.dt.float32
    nc = bacc.Bacc(None)

    f16 = mybir.dt.float16

    xth_d = nc.dram_tensor("xth", [DIN, ROWS], f16, kind="ExternalInput")
    xtl_d = nc.dram_tensor("xtl", [DIN, ROWS], f16, kind="ExternalInput")
    wxh_d = nc.dram_tensor("wxh", [DIN, G4], f16, kind="ExternalInput")
    wxl_d = nc.dram_tensor("wxl", [DIN, G4], f16, kind="ExternalInput")
    whh_d = nc.dram_tensor("whh", [RNN, G4], f16, kind="ExternalInput")
    whl_d = nc.dram_tensor("whl", [RNN, G4], f16, kind="ExternalInput")
    biasw_d = nc.dram_tensor("biasw", [2, G4], f16, kind="ExternalInput")
    smwh_d = nc.dram_tensor("smwh", [RNN, NCC], f16, kind="ExternalInput")
    smwl_d = nc.dram_tensor("smwl", [RNN, NCC], f16, kind="ExternalInput")
    smbb_d = nc.dram_tensor("smbb", [128, NCC], f32, kind="ExternalInput")
    revidx_d = nc.dram_tensor("revidx", [128, NCC], f32, kind="ExternalInput")

    onehot_d = nc.dram_tensor("onehot", [ROWS, NCC], f32, kind="ExternalOutput")
    h_d = nc.dram_tensor("h_out", [BS, RNN], f32, kind="ExternalOutput")
    c_d = nc.dram_tensor("c_out", [BS, RNN], f32, kind="ExternalOutput")

    xth_r = xth_d.rearrange("(kc p) r -> p kc r", p=128)
    xtl_r = xtl_d.rearrange("(kc p) r -> p kc r", p=128)
    wxh_r = wxh_d.rearrange("(kc p) g -> p kc g", p=128)
    wxl_r = wxl_d.rearrange("(kc p) g -> p kc g", p=128)
    whh_r = whh_d.rearrange("(kc p) g -> p kc g", p=128)
    whl_r = whl_d.rearrange("(kc p) g -> p kc g", p=128)
    smwh_r = smwh_d.rearrange("(kc p) n -> p kc n", p=128)
    smwl_r = smwl_d.rearrange("(kc p) n -> p kc n", p=128)

    AluOp = mybir.AluOpType
    Act = mybir.ActivationFunctionType

    with tile.TileContext(nc) as tc:
        with (
            tc.tile_pool(name="persist", bufs=1) as pp,
            tc.tile_pool(name="weights", bufs=1) as wp,
            tc.tile_pool(name="astage", bufs=2) as ap_,
            tc.tile_pool(name="axt", bufs=2) as axt,
            tc.tile_pool(name="bwork", bufs=2) as bp,
            tc.tile_pool(name="bwork1", bufs=1) as bp1,
            tc.tile_pool(name="aps", bufs=3, space="PSUM") as aps,
            tc.tile_pool(name="bps", bufs=1, space="PSUM") as bps,
            tc.tile_pool(name="clps", bufs=1, space="PSUM") as clps,
        ):
            ident = pp.tile([128, 128], f32)
            make_identity(nc, ident[:])
            # h state, transposed, fp16 hi/lo packed along the stationary M
            # dim: HTS[hf][:, kc, s, 0:64] = h_hi^T, [..., 64:128] = h_lo^T.
            # One matmul per (kc, W-pass) computes both hi and lo partial
            # products into psum partition halves (summed by the existing
            # halves-add), so W_hi/W_lo each stream only once per step.
            # Block s=0 is h0 = 0.
            HTS = (pp.tile([128, 2, T + 1, 128], f16, name="hts0"),
                   pp.tile([128, 2, T + 1, 128], f16, name="hts1"))
            for tl in HTS:
                nc.gpsimd.memset(tl[:, :, 0, :], 0.0)

            wxh_t = wp.tile([128, 5, G4], f16)
            wxl_t = wp.tile([128, 5, G4], f16)
            whh_t = wp.tile([128, 4, G4], f16)
            whl_t = wp.tile([128, 4, G4], f16)
            for kc in range(5):
                nc.sync.dma_start(wxh_t[:, kc, :], wxh_r[:, kc, :])
                nc.sync.dma_start(wxl_t[:, kc, :], wxl_r[:, kc, :])
            for kc in range(4):
                nc.sync.dma_start(whh_t[:, kc, :], whh_r[:, kc, :])
                nc.sync.dma_start(whl_t[:, kc, :], whl_r[:, kc, :])
            biasw_t = wp.tile([2, 4, 512], f16)
            nc.sync.dma_start(biasw_t[:], biasw_d.rearrange("k (n x) -> k n x", n=4))
            ones_t = wp.tile([2, 128], f16)
            nc.gpsimd.memset(ones_t[:], 1.0)
            smwh_t = wp.tile([128, 4, NCC], f16)
            nc.sync.dma_start(smwh_t[:], smwh_r[:])
            smwl_t = wp.tile([128, 4, NCC], f16)
            nc.sync.dma_start(smwl_t[:], smwl_r[:])
            smb_t = wp.tile([128, NCC], f32)
            nc.sync.dma_start(smb_t[:], smbb_d[:])
            revidx_t = wp.tile([128, NCC], f32)
            nc.sync.dma_start(revidx_t[:], revidx_d[:])
            logits = pp.tile([128, 16, NCC], f32)

            c_cur = bp.tile([BS, RNN], f32, tag="c")
            nc.gpsimd.memset(c_cur[:], 0.0)

            # Z stage ring: one tile holds one rc-block (128 rows x 4 gates
            # x 512) of the input projection = 2 timesteps worth.  Split into
            # half-blocks (2 gate-chunks) so the PE filler between a step's
            # recurrence matmuls and its transposes stays within PSUM budget;
            # psum->stage copies go on the idle Scalar engine afterwards.
            def emit_phaseA_half_mms(rc, hb, xth_rc, xtl_rc):
                pss = []
                for n in (2 * hb, 2 * hb + 1):
                    ps = aps.tile([128, 512], f32, tag="aps")
                    for kc in range(5):
                        for pi, (lh, rh) in enumerate((
                                (xth_rc, wxh_t), (xth_rc, wxl_t),
                                (xtl_rc, wxh_t))):
                            nc.tensor.matmul(
                                ps[:], lh[:, kc, :], rh[:, kc, bass.ts(n, 512)],
                                start=(kc == 0 and pi == 0), stop=False,
                            )
                    # += bias (hi+lo fp16 rows against a ones stationary)
                    nc.tensor.matmul(ps[:], ones_t[:], biasw_t[:, n, :],
                                     start=False, stop=True)
                    pss.append((n, ps))
                return pss

            def emit_phaseA_half_copies(stage, pss):
                for n, ps in pss:
                    nc.scalar.copy(stage[:, n, :], ps[:])

            def emit_phaseC_block(rc):
                # two 64-row half-blocks; stationary is the packed [hi|lo]
                # state block: psum rows 0:64 = hi terms, 64:128 = lo terms
                for sb in range(2):
                    pl = clps.tile([128, NCC], f32, tag="pl")
                    for kc in range(4):
                        hts = HTS[kc // 2][:, kc % 2, 1 + rc * 2 + sb, :]
                        for qi, rh in enumerate((smwh_t, smwl_t)):
                            nc.tensor.matmul(
                                pl[:], hts, rh[:, kc, :],
                                start=(kc == 0 and qi == 0),
                                stop=(kc == 3 and qi == 1),
                            )
                    out_sl = logits[bass.ds(sb * 64, 64), rc, :]
                    nc.vector.tensor_tensor(out=out_sl, in0=pl[0:64, :],
                                            in1=smb_t[0:64, :], op=AluOp.add)
                    nc.vector.tensor_tensor(out=out_sl, in0=out_sl,
                                            in1=pl[64:128, :], op=AluOp.add)
                rm1 = pp.tile([128, 16], f32, name=f"rm1_{rc}", tag="rm1")
                nc.vector.tensor_reduce(rm1[:, 0:1], logits[:, rc, :],
                                        axis=mybir.AxisListType.X, op=AluOp.max)
                nc.vector.tensor_scalar(out=logits[:, rc, :], in0=logits[:, rc, :],
                                        scalar1=rm1[:, 0:1], scalar2=None,
                                        op0=AluOp.is_equal)
                nc.vector.tensor_tensor(out=logits[:, rc, :], in0=logits[:, rc, :],
                                        in1=revidx_t[:], op=AluOp.mult)
                nc.vector.tensor_reduce(rm1[:, 1:2], logits[:, rc, :],
                                        axis=mybir.AxisListType.X, op=AluOp.max)
                nc.vector.tensor_scalar(out=logits[:, rc, :], in0=revidx_t[:],
                                        scalar1=rm1[:, 1:2], scalar2=None,
                                        op0=AluOp.is_equal)
                nc.sync.dma_start(onehot_d[bass.ts(rc, 128), :], logits[:, rc, :])

            stages = {}
            h2 = None

            def new_ablock(rc):
                xth_rc = axt.tile([128, 5, 128], f16, tag="xth")
                nc.sync.dma_start(xth_rc[:], xth_r[:, :, rc * 128:(rc + 1) * 128])
                xtl_rc = axt.tile([128, 5, 128], f16, tag="xtl")
                nc.sync.dma_start(xtl_rc[:], xtl_r[:, :, rc * 128:(rc + 1) * 128])
                stage = ap_.tile([128, 4, 512], f32, tag="stage")
                return [rc, stage, xth_rc, xtl_rc]

            # prelude: first two rc blocks fully materialized
            for rc in (0, 1):
                blk = new_ablock(rc)
                for hb in range(2):
                    pss = emit_phaseA_half_mms(rc, hb, blk[2], blk[3])
                    emit_phaseA_half_copies(blk[1], pss)
                stages[rc] = blk[1]
            cur_blk = new_ablock(2)
            nxt_blk = new_ablock(3)
            next_half = 0
            pending = None  # (blk, next_half_flag, pss) copies deferred to next step

            C_SCHED = {28: [0, 1, 2, 3], 29: [4, 5, 6, 7],
                       30: [8, 9, 10, 14], 31: [11, 12, 13]}

            for t in range(T):
                # deferred phase-A psum->stage copies from the previous step's
                # filler run early in the Scalar stream, freeing psum before
                # this step's filler matmuls need it
                if pending is not None:
                    pblk, phalf, ppss = pending
                    emit_phaseA_half_copies(pblk[1], ppss)
                    if phalf == 1:
                        stages[pblk[0]] = pblk[1]
                        cur_blk = nxt_blk
                        nxt_blk = (new_ablock(pblk[0] + 2)
                                   if pblk[0] + 2 <= 15 else None)
                        next_half = 0
                    else:
                        next_half = 1
                    pending = None

                rc, half = divmod(t, 2)
                zin = stages[rc][bass.ds(half * BS, BS), :, :]  # [64, 4, 512]

                # ---- recurrence matmuls for step t ----
                # gates: 0=i, 1=f (zpa), 2=g, 3=o (zpb)
                zpa = bps.tile([128, 2, 512], f32, tag="zpa")
                zpb = bps.tile([128, 2, 512], f32, tag="zpb")
                for gp, ps in ((0, zpa), (1, zpb)):
                    for kc in range(4):
                        hts = HTS[kc // 2][:, kc % 2, t, :]
                        for qi, rh in enumerate((whh_t, whl_t)):
                            for ni in range(2):
                                nc.tensor.matmul(
                                    ps[:, ni, :], hts,
                                    rh[:, kc, bass.ts(2 * gp + ni, 512)],
                                    start=(kc == 0 and qi == 0),
                                    stop=(kc == 3 and qi == 1),
                                )

                # ---- PE filler emitted before the pointwise/transposes so the
                # in-order tensor stream works on it while the gate chain runs
                if cur_blk is not None and t >= 2:
                    pending = (cur_blk, next_half,
                               emit_phaseA_half_mms(cur_blk[0], next_half,
                                                    cur_blk[2], cur_blk[3]))
                for crc in C_SCHED.get(t, []):
                    emit_phaseC_block(crc)

                # ---- gate pointwise, h update, transposed fp16 hi/lo state ----
                za = bp.tile([BS, 4, 512], f32, tag="za")
                sif = bp1.tile([BS, 2, 512], f32, tag="sif")
                so = bp1.tile([BS, 512], f32, tag="so")
                m1 = bp1.tile([BS, 512], f32, tag="m1")
                m2 = bp1.tile([BS, 512], f32, tag="m2")
                c_new = bp.tile([BS, RNN], f32, tag="c")
                tc2 = bp1.tile([BS, 512], f32, tag="tc2")
                h2 = bp1.tile([BS, RNN], f32, tag="h2")
                trp = bps.tile([128, 4, 64], f32, tag="zpb")

                for hf in range(2):  # rnn halves: cols hf*256:(hf+1)*256
                    hs = bass.ds(hf * 256, 256)
                    # z' = zp_lo + zin, then += zp_hi  (one PSUM input per op)
                    nc.vector.tensor_tensor(out=za[:, 0:2, hs], in0=zpa[0:64, :, hs],
                                            in1=zin[:, 0:2, hs], op=AluOp.add)
                    nc.vector.tensor_tensor(out=za[:, 0:2, hs], in0=za[:, 0:2, hs],
                                            in1=zpa[64:128, :, hs], op=AluOp.add)
                    nc.vector.tensor_tensor(out=za[:, 2:4, hs], in0=zpb[0:64, :, hs],
                                            in1=zin[:, 2:4, hs], op=AluOp.add)
                    nc.vector.tensor_tensor(out=za[:, 2:4, hs], in0=za[:, 2:4, hs],
                                            in1=zpb[64:128, :, hs], op=AluOp.add)
                    # za = tanh(z'); i/f/o columns pre-scaled by 0.5 on host
                    nc.scalar.activation(za[:, :, hs], za[:, :, hs], Act.Tanh)
                    nc.vector.tensor_scalar(out=sif[:, :, hs], in0=za[:, 0:2, hs],
                                            scalar1=0.5, scalar2=0.5,
                                            op0=AluOp.mult, op1=AluOp.add)
                    nc.gpsimd.tensor_scalar(out=so[:, hs], in0=za[:, 3, hs],
                                            scalar1=0.5, scalar2=0.5,
                                            op0=AluOp.mult, op1=AluOp.add)
                    nc.gpsimd.tensor_tensor(out=m1[:, hs], in0=sif[:, 1, hs],
                                            in1=c_cur[:, hs], op=AluOp.mult)
                    nc.vector.tensor_tensor(out=m2[:, hs], in0=sif[:, 0, hs],
                                            in1=za[:, 2, hs], op=AluOp.mult)
                    nc.vector.tensor_tensor(out=c_new[:, hs], in0=m1[:, hs],
                                            in1=m2[:, hs], op=AluOp.add)
                    nc.scalar.activation(tc2[:, hs], c_new[:, hs], Act.Tanh)
                    nc.vector.tensor_tensor(out=h2[:, hs], in0=so[:, hs],
                                            in1=tc2[:, hs], op=AluOp.mult)
                    # transpose h2 (f32), then split fp16 hi/lo in transposed
                    # space straight into the packed stationary tile
                    for kc in (2 * hf, 2 * hf + 1):
                        nc.tensor.transpose(trp[:, kc, :], h2[:, bass.ts(kc, 128)],
                                            ident[0:64, 0:64])
                    nc.vector.tensor_copy(
                        HTS[hf][:, :, t + 1, 0:64],
                        trp[:, 2 * hf:2 * hf + 2, :])
                    nc.vector.tensor_tensor(
                        out=HTS[hf][:, :, t + 1, 64:128],
                        in0=trp[:, 2 * hf:2 * hf + 2, :],
                        in1=HTS[hf][:, :, t + 1, 0:64], op=AluOp.subtract)
                c_cur = c_new

            if pending is not None:
                emit_phaseA_half_copies(pending[0][1], pending[2])
            emit_phaseC_block(15)
            nc.sync.dma_start(h_d[:], h2[:])
            nc.sync.dma_start(c_d[:], c_cur[:])

    nc.finalize()
    return nc


def _get_program():
    global _PROGRAM
    if _PROGRAM is None:
        _PROGRAM = _build_program()
    return _PROGRAM


def _prep_inputs(f_pool, ground_truth, kernel, rec_kernel, bias, softmax_w,
                 softmax_b):
    def split16(x):
        hi = x.astype(np.float16)
        lo = (x - hi.astype(np.float32)).astype(np.float16)
        return np.ascontiguousarray(hi), np.ascontiguousarray(lo)

    # fold the tanh half-angle pre-scale of gates i, f, o into the weights
    col_scale = np.ones((G4,), np.float32)
    col_scale[0 * RNN:2 * RNN] = 0.5     # i, f
    col_scale[3 * RNN:4 * RNN] = 0.5     # o
    wxh, wxl = split16(kernel * col_scale[None, :])
    whh, whl = split16(rec_kernel * col_scale[None, :])
    bias_s = (bias * col_scale).astype(np.float32)
    bh, bl = split16(bias_s[None, :])
    biasw = np.ascontiguousarray(np.concatenate([bh, bl], axis=0))
    smbb = np.ascontiguousarray(np.tile(softmax_b[None, :].astype(np.float32),
                                        (128, 1)))
    revidx = np.ascontiguousarray(
        np.tile((NCC - np.arange(NCC, dtype=np.float32))[None, :], (128, 1)))
    smwh, smwl = split16(softmax_w.astype(np.float32))

    in_maps = []
    for c in range(NCORES):
        fp = f_pool[c * BS:(c + 1) * BS]          # [64, 32, 512]
        gt = ground_truth[c * BS:(c + 1) * BS]    # [64, 32, 128]
        prev = np.zeros_like(gt)
        prev[:, 1:] = gt[:, :-1]
        fpT = np.ascontiguousarray(fp).transpose(2, 1, 0).reshape(DEPTH, ROWS)
        prT = np.ascontiguousarray(prev).transpose(2, 1, 0).reshape(NCC, ROWS)
        xt = np.concatenate([fpT, prT], axis=0)
        xth, xtl = split16(xt)
        in_maps.append({
            "xth": xth, "xtl": xtl, "wxh": wxh, "wxl": wxl,
            "whh": whh, "whl": whl, "biasw": biasw,
            "smwh": smwh, "smwl": smwl, "smbb": smbb, "revidx": revidx,
        })
    return in_maps


def _install_ntff_shim():
    """Register the axon NTFF profiling hook the image's antenv lacks."""
    import contextlib, ctypes, sys, types
    try:
        import antenv
    except ImportError:
        return
    if getattr(antenv, "axon_hooks", None) is not None:
        return
    state = {}
    mod = types.ModuleType("antenv.axon_hooks")
    mod.set_axon_ntff_profile_hook = lambda h: state.update(h=h)
    mod.get_axon_ntff_profile_hook = lambda: state.get("h")
    sys.modules["antenv.axon_hooks"] = mod
    antenv.axon_hooks = mod
    try:
        lib = ctypes.CDLL("/opt/axon/libaxon_pjrt.so")
    except OSError:
        return
    if not hasattr(lib, "axon_start_nrt_profile"):
        return
    lib.axon_start_nrt_profile.argtypes = [ctypes.POINTER(ctypes.c_int64),
                                           ctypes.c_size_t]
    lib.axon_start_nrt_profile.restype = ctypes.c_int64
    lib.axon_stop_nrt_profile.argtypes = [ctypes.c_char_p]
    lib.axon_stop_nrt_profile.restype = ctypes.c_int64

    @contextlib.contextmanager
    def _hook(output_dir, device_ids):
        import jax
        jax.devices()
        if device_ids:
            ids = (ctypes.c_int64 * len(device_ids))(*device_ids)
            rc = lib.axon_start_nrt_profile(ids, len(device_ids))
        else:
            rc = lib.axon_start_nrt_profile(None, 0)
        if rc != 0:
            raise RuntimeError(f"axon_start_nrt_profile rc={rc}")
        try:
            yield
        finally:
            n = lib.axon_stop_nrt_profile(str(output_dir).encode())
            if n < 0:
                raise RuntimeError(f"axon_stop_nrt_profile rc={n}")

    mod.set_axon_ntff_profile_hook(_hook)


def _run(in_maps, trace=False):
    from concourse.bass_utils import run_bass_kernel_spmd
    if trace:
        _install_ntff_shim()
    nc = _get_program()
    return run_bass_kernel_spmd(nc, in_maps, list(range(NCORES)), trace=trace)


def kernel(f_pool, ground_truth, kernel, rec_kernel, bias, softmax_w,
           softmax_b):
    f_pool = np.asarray(f_pool, np.float32)
    ground_truth = np.asarray(ground_truth, np.float32)
    in_maps = _prep_inputs(f_pool, ground_truth, np.asarray(kernel, np.float32),
                           np.asarray(rec_kernel, np.float32),
                           np.asarray(bias, np.float32),
                           np.asarray(softmax_w, np.float32),
                           np.asarray(softmax_b, np.float32))
    trace = bool(int(os.environ.get("KERNEL_TRACE", "0")))
    res = _run(in_maps, trace=trace)
    if trace and res.exec_time_ns is not None:
        print(f"HW exec time: {res.exec_time_ns} ns")

    seq = np.empty((B, T, NCC), np.float32)
    h = np.empty((B, RNN), np.float32)
    c = np.empty((B, RNN), np.float32)
    for ci in range(NCORES):
        r = res.results[ci]
        seq[ci * BS:(ci + 1) * BS] = (
            r["onehot"].reshape(T, BS, NCC).transpose(1, 0, 2))
        h[ci * BS:(ci + 1) * BS] = r["h_out"]
        c[ci * BS:(ci + 1) * BS] = r["c_out"]
    return (seq, h, c)


# revision 35
# speedup vs baseline: 1.0997x; 1.0041x over previous
"""ChaRNN LSTM (teacher forcing) Trainium2 Bass kernel.

Structure (data-parallel over batch, 64 rows/core on 8 cores):
  - Teacher forcing means the LSTM input at step t is [f_pool[:,t,:], gt[:,t-1,:]]
    which is fully known ahead of time, so the input projection X @ W_x for all
    32 steps is a big batched matmul (phase A).  Only h @ W_h is sequential
    (phase B).  The argmax/one-hot head is deferred and batched (phase C).
  - Matmuls use fp16 hi/lo 3-pass splits (3 cyc/row total, fp32-grade\n    accuracy, validated 0 argmax flips) - single-pass bf16/fp16 flips
    argmaxes (one-hot output is graded, top-2 logit gaps go down to 6.6e-6).
  - Gate activations use tanh only (4 ULP) via sigmoid(x) = 0.5*tanh(x/2)+0.5;
    the 0.5 pre-scale of the i/f/o gate columns is folded into the weights on
    the host, so one ACT pass computes tanh over the whole 2048-wide gate row.
  - Phase A is emitted interleaved with the recurrence so the tensor engine
    fills the pointwise-tail gaps of each step with input-projection matmuls.
  - Recurrence matmuls pack pairs of K-chunks into PE column halves
    (tile_position (0,0)/(0,64)) since batch=64 only fills half the array;
    measured 1.84x over the unpacked form.
"""

import os
import numpy as np

B, T, DEPTH = 512, 32, 512
RNN, NCC = 512, 128
DIN = DEPTH + NCC            # 640
G4 = 4 * RNN                 # 2048
NCORES = 8
BS = B // NCORES             # 64 batch rows per core
ROWS = T * BS                # 2048 (t-major: r = t*BS + b)

_PROGRAM = None


def _build_program():
    import concourse.bass as bass
    import concourse.tile as tile
    from concourse import bacc, mybir
    from concourse.masks import make_identity

    f32 = mybir.dt.float32
    nc = bacc.Bacc(None)

    f16 = mybir.dt.float16

    xth_d = nc.dram_tensor("xth", [DIN, ROWS], f16, kind="ExternalInput")
    xtl_d = nc.dram_tensor("xtl", [DIN, ROWS], f16, kind="ExternalInput")
    wxh_d = nc.dram_tensor("wxh", [DIN, G4], f16, kind="ExternalInput")
    wxl_d = nc.dram_tensor("wxl", [DIN, G4], f16, kind="ExternalInput")
    whh_d = nc.dram_tensor("whh", [RNN, G4], f16, kind="ExternalInput")
    whl_d = nc.dram_tensor("whl", [RNN, G4], f16, kind="ExternalInput")
    biasw_d = nc.dram_tensor("biasw", [2, G4], f16, kind="ExternalInput")
    smwh_d = nc.dram_tensor("smwh", [RNN, NCC], f16, kind="ExternalInput")
    smwl_d = nc.dram_tensor("smwl", [RNN, NCC], f16, kind="ExternalInput")
    smbb_d = nc.dram_tensor("smbb", [128, NCC], f32, kind="ExternalInput")
    revidx_d = nc.dram_tensor("revidx", [128, NCC], f32, kind="ExternalInput")

    onehot_d = nc.dram_tensor("onehot", [ROWS, NCC], f32, kind="ExternalOutput")
    h_d = nc.dram_tensor("h_out", [BS, RNN], f32, kind="ExternalOutput")
    c_d = nc.dram_tensor("c_out", [BS, RNN], f32, kind="ExternalOutput")

    xth_r = xth_d.rearrange("(kc p) r -> p kc r", p=128)
    xtl_r = xtl_d.rearrange("(kc p) r -> p kc r", p=128)
    wxh_r = wxh_d.rearrange("(kc p) g -> p kc g", p=128)
    wxl_r = wxl_d.rearrange("(kc p) g -> p kc g", p=128)
    whh_r = whh_d.rearrange("(kc p) g -> p kc g", p=128)
    whl_r = whl_d.rearrange("(kc p) g -> p kc g", p=128)
    smwh_r = smwh_d.rearrange("(kc p) n -> p kc n", p=128)
    smwl_r = smwl_d.rearrange("(kc p) n -> p kc n", p=128)

    AluOp = mybir.AluOpType
    Act = mybir.ActivationFunctionType

    with tile.TileContext(nc) as tc:
        with (
            tc.tile_pool(name="persist", bufs=1) as pp,
            tc.tile_pool(name="weights", bufs=1) as wp,
            tc.tile_pool(name="astage", bufs=2) as ap_,
            tc.tile_pool(name="axt", bufs=2) as axt,
            tc.tile_pool(name="bwork", bufs=2) as bp,
            tc.tile_pool(name="bwork1", bufs=1) as bp1,
            tc.tile_pool(name="aps", bufs=3, space="PSUM") as aps,
            tc.tile_pool(name="bps", bufs=1, space="PSUM") as bps,
            tc.tile_pool(name="clps", bufs=1, space="PSUM") as clps,
        ):
            ident = pp.tile([128, 128], f32)
            make_identity(nc, ident[:])
            # h state, transposed, fp16 hi/lo packed along the stationary M
            # dim: HTS[hf][:, kc, s, 0:64] = h_hi^T, [..., 64:128] = h_lo^T.
            # One matmul per (kc, W-pass) computes both hi and lo partial
            # products into psum partition halves (summed by the existing
            # halves-add), so W_hi/W_lo each stream only once per step.
            # Block s=0 is h0 = 0.
            HTS = (pp.tile([128, 2, T + 1, 128], f16, name="hts0"),
                   pp.tile([128, 2, T + 1, 128], f16, name="hts1"))
            for tl in HTS:
                nc.gpsimd.memset(tl[:, :, 0, :], 0.0)

            wxh_t = wp.tile([128, 5, G4], f16)
            wxl_t = wp.tile([128, 5, G4], f16)
            whh_t = wp.tile([128, 4, G4], f16)
            whl_t = wp.tile([128, 4, G4], f16)
            for kc in range(5):
                nc.sync.dma_start(wxh_t[:, kc, :], wxh_r[:, kc, :])
                nc.sync.dma_start(wxl_t[:, kc, :], wxl_r[:, kc, :])
            for kc in range(4):
                nc.sync.dma_start(whh_t[:, kc, :], whh_r[:, kc, :])
                nc.sync.dma_start(whl_t[:, kc, :], whl_r[:, kc, :])
            biasw_t = wp.tile([2, 4, 512], f16)
            nc.sync.dma_start(biasw_t[:], biasw_d.rearrange("k (n x) -> k n x", n=4))
            ones_t = wp.tile([2, 128], f16)
            nc.gpsimd.memset(ones_t[:], 1.0)
            smwh_t = wp.tile([128, 4, NCC], f16)
            nc.sync.dma_start(smwh_t[:], smwh_r[:])
            smwl_t = wp.tile([128, 4, NCC], f16)
            nc.sync.dma_start(smwl_t[:], smwl_r[:])
            smb_t = wp.tile([128, NCC], f32)
            nc.sync.dma_start(smb_t[:], smbb_d[:])
            revidx_t = wp.tile([128, NCC], f32)
            nc.sync.dma_start(revidx_t[:], revidx_d[:])
            logits = pp.tile([128, 16, NCC], f32)

            c_cur = bp.tile([BS, RNN], f32, tag="c")
            nc.gpsimd.memset(c_cur[:], 0.0)

            # Z stage ring: one tile holds one rc-block (128 rows x 4 gates
            # x 512) of the input projection = 2 timesteps worth.  Split into
            # half-blocks (2 gate-chunks) so the PE filler between a step's
            # recurrence matmuls and its transposes stays within PSUM budget;
            # psum->stage copies go on the idle Scalar engine afterwards.
            def emit_phaseA_half_mms(rc, hb, xth_rc, xtl_rc):
                pss = []
                for n in (2 * hb, 2 * hb + 1):
                    ps = aps.tile([128, 512], f32, tag="aps")
                    # bias first (hi+lo fp16 rows against a ones stationary):
                    # it has no input dependencies, so it can start the group
                    # while the xt chunk DMA is still in flight
                    nc.tensor.matmul(ps[:], ones_t[:], biasw_t[:, n, :],
                                     start=True, stop=False)
                    for kc in range(5):
                        for pi, (lh, rh) in enumerate((
                                (xth_rc, wxh_t), (xth_rc, wxl_t),
                                (xtl_rc, wxh_t))):
                            nc.tensor.matmul(
                                ps[:], lh[:, kc, :], rh[:, kc, bass.ts(n, 512)],
                                start=False, stop=(kc == 4 and pi == 2),
                            )
                    pss.append((n, ps))
                return pss

            def emit_phaseA_half_copies(stage, pss):
                for n, ps in pss:
                    nc.scalar.copy(stage[:, n, :], ps[:])

            def emit_phaseC_block(rc):
                # two 64-row half-blocks; stationary is the packed [hi|lo]
                # state block: psum rows 0:64 = hi terms, 64:128 = lo terms
                for sb in range(2):
                    pl = clps.tile([128, NCC], f32, tag="pl")
                    for kc in range(4):
                        hts = HTS[kc // 2][:, kc % 2, 1 + rc * 2 + sb, :]
                        for qi, rh in enumerate((smwh_t, smwl_t)):
                            nc.tensor.matmul(
                                pl[:], hts, rh[:, kc, :],
                                start=(kc == 0 and qi == 0),
                                stop=(kc == 3 and qi == 1),
                            )
                    out_sl = logits[bass.ds(sb * 64, 64), rc, :]
                    nc.vector.tensor_tensor(out=out_sl, in0=pl[0:64, :],
                                            in1=smb_t[0:64, :], op=AluOp.add)
                    nc.vector.tensor_tensor(out=out_sl, in0=out_sl,
                                            in1=pl[64:128, :], op=AluOp.add)
                rm1 = pp.tile([128, 16], f32, name=f"rm1_{rc}", tag="rm1")
                nc.vector.tensor_reduce(rm1[:, 0:1], logits[:, rc, :],
                                        axis=mybir.AxisListType.X, op=AluOp.max)
                nc.vector.tensor_scalar(out=logits[:, rc, :], in0=logits[:, rc, :],
                                        scalar1=rm1[:, 0:1], scalar2=None,
                                        op0=AluOp.is_equal)
                nc.vector.tensor_tensor(out=logits[:, rc, :], in0=logits[:, rc, :],
                                        in1=revidx_t[:], op=AluOp.mult)
                nc.vector.tensor_reduce(rm1[:, 1:2], logits[:, rc, :],
                                        axis=mybir.AxisListType.X, op=AluOp.max)
                nc.vector.tensor_scalar(out=logits[:, rc, :], in0=revidx_t[:],
                                        scalar1=rm1[:, 1:2], scalar2=None,
                                        op0=AluOp.is_equal)
                nc.sync.dma_start(onehot_d[bass.ts(rc, 128), :], logits[:, rc, :])

            stages = {}
            h2 = None

            def new_ablock(rc):
                xth_rc = axt.tile([128, 5, 128], f16, tag="xth")
                nc.sync.dma_start(xth_rc[:], xth_r[:, :, rc * 128:(rc + 1) * 128])
                xtl_rc = axt.tile([128, 5, 128], f16, tag="xtl")
                nc.sync.dma_start(xtl_rc[:], xtl_r[:, :, rc * 128:(rc + 1) * 128])
                stage = ap_.tile([128, 4, 512], f32, tag="stage")
                return [rc, stage, xth_rc, xtl_rc]

            # prelude: first two rc blocks fully materialized
            for rc in (0, 1):
                blk = new_ablock(rc)
                for hb in range(2):
                    pss = emit_phaseA_half_mms(rc, hb, blk[2], blk[3])
                    emit_phaseA_half_copies(blk[1], pss)
                stages[rc] = blk[1]
            cur_blk = new_ablock(2)
            nxt_blk = new_ablock(3)
            next_half = 0
            pending = None  # (blk, next_half_flag, pss) copies deferred to next step

            C_SCHED = {28: [0, 1, 2, 3], 29: [4, 5, 6, 7],
                       30: [8, 9, 10, 14], 31: [11, 12, 13]}

            for t in range(T):
                # deferred phase-A psum->stage copies from the previous step's
                # filler run early in the Scalar stream, freeing psum before
                # this step's filler matmuls need it
                if pending is not None:
                    pblk, phalf, ppss = pending
                    emit_phaseA_half_copies(pblk[1], ppss)
                    if phalf == 1:
                        stages[pblk[0]] = pblk[1]
                        cur_blk = nxt_blk
                        nxt_blk = (new_ablock(pblk[0] + 2)
                                   if pblk[0] + 2 <= 15 else None)
                        next_half = 0
                    else:
                        next_half = 1
                    pending = None

                rc, half = divmod(t, 2)
                zin = stages[rc][bass.ds(half * BS, BS), :, :]  # [64, 4, 512]

                # ---- recurrence matmuls for step t ----
                # gates: 0=i, 1=f (zpa), 2=g, 3=o (zpb)
                zpa = bps.tile([128, 2, 512], f32, tag="zpa")
                zpb = bps.tile([128, 2, 512], f32, tag="zpb")
                for gp, ps in ((0, zpa), (1, zpb)):
                    for kc in range(4):
                        hts = HTS[kc // 2][:, kc % 2, t, :]
                        for qi, rh in enumerate((whh_t, whl_t)):
                            for ni in range(2):
                                nc.tensor.matmul(
                                    ps[:, ni, :], hts,
                                    rh[:, kc, bass.ts(2 * gp + ni, 512)],
                                    start=(kc == 0 and qi == 0),
                                    stop=(kc == 3 and qi == 1),
                                )

                # ---- PE filler emitted before the pointwise/transposes so the
                # in-order tensor stream works on it while the gate chain runs
                if cur_blk is not None and t >= 2:
                    pending = (cur_blk, next_half,
                               emit_phaseA_half_mms(cur_blk[0], next_half,
                                                    cur_blk[2], cur_blk[3]))
                for crc in C_SCHED.get(t, []):
                    emit_phaseC_block(crc)

                # ---- gate pointwise, h update, transposed fp16 hi/lo state ----
                za = bp.tile([BS, 4, 512], f32, tag="za")
                sif = bp1.tile([BS, 2, 512], f32, tag="sif")
                so = bp1.tile([BS, 512], f32, tag="so")
                m1 = bp1.tile([BS, 512], f32, tag="m1")
                m2 = bp1.tile([BS, 512], f32, tag="m2")
                c_new = bp.tile([BS, RNN], f32, tag="c")
                tc2 = bp1.tile([BS, 512], f32, tag="tc2")
                h2 = bp1.tile([BS, RNN], f32, tag="h2")
                trp = bps.tile([128, 4, 64], f32, tag="zpb")

                for hf in range(2):  # rnn halves: cols hf*256:(hf+1)*256
                    hs = bass.ds(hf * 256, 256)
                    # z' = zp_lo + zin, then += zp_hi  (one PSUM input per op)
                    nc.vector.tensor_tensor(out=za[:, 0:2, hs], in0=zpa[0:64, :, hs],
                                            in1=zin[:, 0:2, hs], op=AluOp.add)
                    nc.vector.tensor_tensor(out=za[:, 0:2, hs], in0=za[:, 0:2, hs],
                                            in1=zpa[64:128, :, hs], op=AluOp.add)
                    nc.vector.tensor_tensor(out=za[:, 2:4, hs], in0=zpb[0:64, :, hs],
                                            in1=zin[:, 2:4, hs], op=AluOp.add)
                    nc.vector.tensor_tensor(out=za[:, 2:4, hs], in0=za[:, 2:4, hs],
                                            in1=zpb[64:128, :, hs], op=AluOp.add)
                    # za = tanh(z'); i/f/o columns pre-scaled by 0.5 on host
                    nc.scalar.activation(za[:, :, hs], za[:, :, hs], Act.Tanh)
                    nc.vector.tensor_scalar(out=sif[:, :, hs], in0=za[:, 0:2, hs],
                                            scalar1=0.5, scalar2=0.5,
                                            op0=AluOp.mult, op1=AluOp.add)
                    nc.gpsimd.tensor_scalar(out=so[:, hs], in0=za[:, 3, hs],
                                            scalar1=0.5, scalar2=0.5,
                                            op0=AluOp.mult, op1=AluOp.add)
                    nc.gpsimd.tensor_tensor(out=m1[:, hs], in0=sif[:, 1, hs],
                                            in1=c_cur[:, hs], op=AluOp.mult)
                    nc.vector.tensor_tensor(out=m2[:, hs], in0=sif[:, 0, hs],
                                            in1=za[:, 2, hs], op=AluOp.mult)
                    nc.vector.tensor_tensor(out=c_new[:, hs], in0=m1[:, hs],
                                            in1=m2[:, hs], op=AluOp.add)
                    nc.scalar.activation(tc2[:, hs], c_new[:, hs], Act.Tanh)
                    nc.vector.tensor_tensor(out=h2[:, hs], in0=so[:, hs],
                                            in1=tc2[:, hs], op=AluOp.mult)
                    # transpose h2 (f32), then split fp16 hi/lo in transposed
                    # space straight into the packed stationary tile
                    for kc in (2 * hf, 2 * hf + 1):
                        nc.tensor.transpose(trp[:, kc, :], h2[:, bass.ts(kc, 128)],
                                            ident[0:64, 0:64])
                    nc.vector.tensor_copy(
                        HTS[hf][:, :, t + 1, 0:64],
                        trp[:, 2 * hf:2 * hf + 2, :])
                    nc.vector.tensor_tensor(
                        out=HTS[hf][:, :, t + 1, 64:128],
                        in0=trp[:, 2 * hf:2 * hf + 2, :],
                        in1=HTS[hf][:, :, t + 1, 0:64], op=AluOp.subtract)
                c_cur = c_new

            if pending is not None:
                emit_phaseA_half_copies(pending[0][1], pending[2])
            emit_phaseC_block(15)
            nc.sync.dma_start(h_d[:], h2[:])
            nc.sync.dma_start(c_d[:], c_cur[:])

    nc.finalize()
    return nc


def _get_program():
    global _PROGRAM
    if _PROGRAM is None:
        _PROGRAM = _build_program()
    return _PROGRAM


def _prep_inputs(f_pool, ground_truth, kernel, rec_kernel, bias, softmax_w,
                 softmax_b):
    def split16(x):
        hi = x.astype(np.float16)
        lo = (x - hi.astype(np.float32)).astype(np.float16)
        return np.ascontiguousarray(hi), np.ascontiguousarray(lo)

    # fold the tanh half-angle pre-scale of gates i, f, o into the weights
    col_scale = np.ones((G4,), np.float32)
    col_scale[0 * RNN:2 * RNN] = 0.5     # i, f
    col_scale[3 * RNN:4 * RNN] = 0.5     # o
    wxh, wxl = split16(kernel * col_scale[None, :])
    whh, whl = split16(rec_kernel * col_scale[None, :])
    bias_s = (bias * col_scale).astype(np.float32)
    bh, bl = split16(bias_s[None, :])
    biasw = np.ascontiguousarray(np.concatenate([bh, bl], axis=0))
    smbb = np.ascontiguousarray(np.tile(softmax_b[None, :].astype(np.float32),
                                        (128, 1)))
    revidx = np.ascontiguousarray(
        np.tile((NCC - np.arange(NCC, dtype=np.float32))[None, :], (128, 1)))
    smwh, smwl = split16(softmax_w.astype(np.float32))

    in_maps = []
    for c in range(NCORES):
        fp = f_pool[c * BS:(c + 1) * BS]          # [64, 32, 512]
        gt = ground_truth[c * BS:(c + 1) * BS]    # [64, 32, 128]
        prev = np.zeros_like(gt)
        prev[:, 1:] = gt[:, :-1]
        fpT = np.ascontiguousarray(fp).transpose(2, 1, 0).reshape(DEPTH, ROWS)
        prT = np.ascontiguousarray(prev).transpose(2, 1, 0).reshape(NCC, ROWS)
        xt = np.concatenate([fpT, prT], axis=0)
        xth, xtl = split16(xt)
        in_maps.append({
            "xth": xth, "xtl": xtl, "wxh": wxh, "wxl": wxl,
            "whh": whh, "whl": whl, "biasw": biasw,
            "smwh": smwh, "smwl": smwl, "smbb": smbb, "revidx": revidx,
        })
    return in_maps


def _install_ntff_shim():
    """Register the axon NTFF profiling hook the image's antenv lacks."""
    import contextlib, ctypes, sys, types
    try:
        import antenv
    except ImportError:
        return
    if getattr(antenv, "axon_hooks", None) is not None:
        return
    state = {}
    mod = types.ModuleType("antenv.axon_hooks")
    mod.set_axon_ntff_profile_hook = lambda h: state.update(h=h)
    mod.get_axon_ntff_profile_hook = lambda: state.get("h")
    sys.modules["antenv.axon_hooks"] = mod
    antenv.axon_hooks = mod
    try:
        lib = ctypes.CDLL("/opt/axon/libaxon_pjrt.so")
    except OSError:
        return
    if not hasattr(lib, "axon_start_nrt_profile"):
        return
    lib.axon_start_nrt_profile.argtypes = [ctypes.POINTER(ctypes.c_int64),
                                           ctypes.c_size_t]
    lib.axon_start_nrt_profile.restype = ctypes.c_int64
    lib.axon_stop_nrt_profile.argtypes = [ctypes.c_char_p]
    lib.axon_stop_nrt_profile.restype = ctypes.c_int64

    @contextlib.contextmanager
    def _hook(output_dir, device_ids):
        import jax
        jax.devices()
        if device_ids:
            ids = (ctypes.c_int64 * len(device_ids))(*device_ids)
            rc = lib.axon_start_nrt_profile(ids, len(device_ids))
        else:
            rc = lib.axon_start_nrt_profile(None, 0)
        if rc != 0:
            raise RuntimeError(f"axon_start_nrt_profile rc={rc}")
        try:
            yield
        finally:
            n = lib.axon_stop_nrt_profile(str(output_dir).encode())
            if n < 0:
                raise RuntimeError(f"axon_stop_nrt_profile rc={n}")

    mod.set_axon_ntff_profile_hook(_hook)


def _run(in_maps, trace=False):
    from concourse.bass_utils import run_bass_kernel_spmd
    if trace:
        _install_ntff_shim()
    nc = _get_program()
    return run_bass_kernel_spmd(nc, in_maps, list(range(NCORES)), trace=trace)


def kernel(f_pool, ground_truth, kernel, rec_kernel, bias, softmax_w,
           softmax_b):
    f_pool = np.asarray(f_pool, np.float32)
    ground_truth = np.asarray(ground_truth, np.float32)
    in_maps = _prep_inputs(f_pool, ground_truth, np.asarray(kernel, np.float32),
                           np.asarray(rec_kernel, np.float32),
                           np.asarray(bias, np.float32),
                           np.asarray(softmax_w, np.float32),
                           np.asarray(softmax_b, np.float32))
    trace = bool(int(os.environ.get("KERNEL_TRACE", "0")))
    res = _run(in_maps, trace=trace)
    if trace and res.exec_time_ns is not None:
        print(f"HW exec time: {res.exec_time_ns} ns")

    seq = np.empty((B, T, NCC), np.float32)
    h = np.empty((B, RNN), np.float32)
    c = np.empty((B, RNN), np.float32)
    for ci in range(NCORES):
        r = res.results[ci]
        seq[ci * BS:(ci + 1) * BS] = (
            r["onehot"].reshape(T, BS, NCC).transpose(1, 0, 2))
        h[ci * BS:(ci + 1) * BS] = r["h_out"]
        c[ci * BS:(ci + 1) * BS] = r["c_out"]
    return (seq, h, c)
